# revision 2
# baseline (speedup 1.0000x reference)
import sys

sys.path.insert(0, "/opt/trn_rl_repo")

import numpy as np

# Problem constants (hardcoded per spec nn_AdaptivePriorBoxesLoss)
P_TOT = 131072
T = 256
NCORES = 8
PL = P_TOT // NCORES  # 16384 priors per core
ROWS = 128
NT = PL // ROWS  # 128 tiles per core; local prior p = q*NT + j
TH_F = 2.0 / 7.0  # iou > 0.4  <=>  inter/(areaA+areaB) > 2/7
K_VAL = 2.5
BETA = 1.0

_CACHE = {}


# ---------------------------------------------------------------------------
# Custom DVE ops (registered into concourse.dve_ops process-locally)
# ---------------------------------------------------------------------------
def _register_custom_ops():
    from concourse import dve_ops
    from concourse.dve_ops import DveOp
    from concourse.dve_spec import (
        Spec,
        Src0,
        Src1,
        C0,
        C1,
        C2,
        Zero,
        relu,
        minn,
        maxx,
        lower,
        _has_src1,
    )
    from concourse.dve_uop import DveOpSpec
    from operator import add as _add

    def mk(name, spec):
        if name in dve_ops._SUB_OPCODE_FOR_NAME:
            for op in dve_ops.OPS:
                if op.name == name:
                    return op
        row = dve_ops._CUSTOM_DVE_ROW_BASE + len(dve_ops.OPS)
        shas = {}
        for ver in ("v3", "v4"):
            u = lower(spec, ver=ver)
            ds = DveOpSpec(name=name, opcode=row, uops=u, rd1_en=_has_src1(spec))
            shas[ver] = ds.sha(ver)
        op = DveOp(name, spec, subdim=False, uops_sha=shas)
        dve_ops.OPS.append(op)
        dve_ops._SUB_OPCODE_FOR_NAME[name] = row
        dve_ops.CUSTOM_DVE_SPECS[name] = spec
        return op

    # interval overlap: out = relu(min(Src0 - s0, s1) - relu(Src1 - s0))
    #  = relu(min(tx2 - px1, pw) - relu(tx1 - px1))  [per-truth free dim]
    def _ov_ref(in0, in1, s0, s1, imm2):
        a = np.minimum(in0.astype(np.float32) - s0, s1)
        b = np.maximum(in1.astype(np.float32) - s0, 0.0)
        return np.maximum(a - b, 0.0).astype(np.float32)

    ov = mk(
        "OVERLAP_LEN_ANT",
        Spec(body=relu(minn(Src0 - C0, C1) - relu(Src1 - C0)), reference=_ov_ref),
    )

    # flipped roles (tensor = prior arrays, scalars = truth coords):
    # out = relu(min(s0 - Src0, Src1) - relu(s1 - Src0))
    def _ovf_ref(in0, in1, s0, s1, imm2):
        a = np.minimum(s0 - in0.astype(np.float32), in1)
        b = np.maximum(s1 - in0.astype(np.float32), 0.0)
        return np.maximum(a - b, 0.0).astype(np.float32)

    ovf = mk(
        "OVERLAP_FLIP_ANT",
        Spec(body=relu(minn(C0 - Src0, Src1) - relu(C1 - Src0)), reference=_ovf_ref),
    )

    # fused elementwise product + running-max accumulation
    def _fm_ref(in0, in1, s0, s1, imm2):
        b = (in0.astype(np.float32) * in1).astype(np.float32)
        return b, np.maximum(
            b.reshape(b.shape[0], -1).max(axis=-1, keepdims=True),
            np.asarray(s0, np.float32).reshape(-1, 1),
        )

    fm = mk(
        "FMUL_MAX_ANT",
        Spec(body=Src0 * Src1, accum=maxx, accum_init=C0, reference=_fm_ref),
    )

    # smooth-l1 core: a=|x|; out = min(a,s0)*a + relu(a-s0)   (s0=1.0)
    def _sl1_ref(in0, in1, s0, s1, imm2):
        aa = np.abs(in0.astype(np.float32))
        return (np.minimum(aa, s0) * aa + np.maximum(aa - s0, 0.0)).astype(np.float32)

    a_expr = maxx(Src0, Zero - Src0)
    sl1 = mk(
        "SMOOTHL1_ANT",
        Spec(body=minn(a_expr, C0) * a_expr + relu(a_expr - C0), reference=_sl1_ref),
    )

    # fused final product: out = relu(in0) * relu(in1) * s0, with running-max
    # accumulation (seeded from s1) into accum_out
    def _fmr_ref(in0, in1, s0, s1, imm2):
        b = (
            np.maximum(in0.astype(np.float32), 0.0)
            * np.maximum(in1.astype(np.float32), 0.0)
            * np.asarray(s0, np.float32).reshape(-1, 1)
        ).astype(np.float32)
        return b, np.maximum(
            b.reshape(b.shape[0], -1).max(axis=-1, keepdims=True),
            np.asarray(s1, np.float32).reshape(-1, 1),
        )

    fmr = mk(
        "FMULRELU_MAX_ANT",
        Spec(
            body=relu(Src0) * relu(Src1) * C0,
            accum=maxx,
            accum_init=C1,
            reference=_fmr_ref,
        ),
    )

    # f = relu(in0) * relu(s1 - in1) * s0, running-max accum (init imm2)
    def _fmr3_ref(in0, in1, s0, s1, imm2):
        b = (
            np.maximum(in0.astype(np.float32), 0.0)
            * np.maximum(
                np.asarray(s1, np.float32).reshape(-1, 1) - in1.astype(np.float32),
                0.0,
            )
            * np.asarray(s0, np.float32).reshape(-1, 1)
        ).astype(np.float32)
        return b, np.maximum(
            b.reshape(b.shape[0], -1).max(axis=-1, keepdims=True),
            np.asarray(imm2, np.float32),
        )

    fmr3 = mk(
        "FMR3_ANT",
        Spec(
            body=relu(Src0) * relu(C1 - Src1) * C0,
            accum=maxx,
            accum_init=C2,
            reference=_fmr3_ref,
        ),
    )
    return ov, ovf, fm, sl1, fmr, fmr3


def _raw_activation(nc, out, in_, func, bias_imm, scale_ap):
    """nc.scalar.activation minus the Reciprocal guard: out = func(in*scale+bias).
    bias is a float immediate (required for Reciprocal), scale a [P,1] AP."""
    from concourse import mybir

    inputs = [
        nc.scalar.lower_ap(in_),
        mybir.ImmediateValue(dtype=mybir.dt.float32, value=float(bias_imm)),
        nc.scalar.lower_ap(scale_ap),
        mybir.ImmediateValue(dtype=mybir.dt.float32, value=0.0),
    ]
    return nc.scalar.add_instruction(
        mybir.InstActivation(
            name=nc.get_next_instruction_name(),
            func=func,
            ins=inputs,
            outs=[nc.scalar.lower_ap(out)],
        )
    )


def _build():
    from concourse import bass, mybir, tile
    from concourse.masks import make_identity

    OV_OP, OVF_OP, FM_OP, SL1_OP, FMR_OP, FMR3_OP = _register_custom_ops()

    f32 = mybir.dt.float32
    f16 = mybir.dt.float16
    i32 = mybir.dt.int32
    u32 = mybir.dt.uint32
    AF = mybir.ActivationFunctionType
    OP = mybir.AluOpType

    nc = bass.Bass()
    locs_ext = nc.declare_dram_parameter("locs", [PL, 2], f32, isOutput=False)
    params_ext = nc.declare_dram_parameter("params", [PL, 3], f32, isOutput=False)
    truths4_ext = nc.declare_dram_parameter("truths4", [4, T], f32, isOutput=False)
    poff_ext = nc.declare_dram_parameter("poff", [1, 1], f32, isOutput=False)
    out_ext = nc.declare_dram_parameter("out", [1, 1], f32, isOutput=True)

    RG = [list(range(NCORES))]

    with tile.TileContext(nc) as tc:
        with (
            tc.tile_pool(name="persist", bufs=1) as pe,
            tc.tile_pool(name="work", bufs=8) as wp,
            tc.tile_pool(name="psum", bufs=2, space="PSUM") as pp,
            tc.tile_pool(name="dram", bufs=1, space="DRAM") as dp,
        ):
            # ---------------- Phase 0: prep ----------------
            locs_sb = pe.tile([ROWS, NT, 2], f32, name="locs_sb")
            params_sb = pe.tile([ROWS, NT, 3], f32, name="params_sb")
            nc.sync.dma_start(
                out=locs_sb[:], in_=locs_ext[:].rearrange("(r j) c -> r j c", r=ROWS)
            )
            nc.sync.dma_start(
                out=params_sb[:],
                in_=params_ext[:].rearrange("(r j) c -> r j c", r=ROWS),
            )

            ones128 = pe.tile([ROWS, 1], f32, name="ones128")
            nc.vector.memset(ones128[:], 1.0)
            ident = pe.tile([ROWS, ROWS], f32, name="ident")
            make_identity(nc, ident[:])

            # sigmoid first: keeps the ACT table switches to 2 total
            s_sb = pe.tile([ROWS, NT], f32, name="s_sb")
            nc.scalar.activation(
                out=s_sb[:], in_=params_sb[:, :, 2], func=AF.Sigmoid
            )

            # truth broadcast tiles (f32) via row-broadcast DMA
            TX1b = pe.tile([ROWS, T], f32, name="TX1b")
            TY1b = pe.tile([ROWS, T], f32, name="TY1b")
            TX2b = pe.tile([ROWS, T], f32, name="TX2b")
            TY2b = pe.tile([ROWS, T], f32, name="TY2b")
            for rix, bt in enumerate((TX1b, TY1b, TX2b, TY2b)):
                nc.sync.dma_start(
                    out=bt[:], in_=truths4_ext[rix : rix + 1, :].to_broadcast([ROWS, T])
                )
            POFFb = pe.tile([ROWS, 1], f32, name="POFFb")
            nc.sync.dma_start(out=POFFb[:], in_=poff_ext[:].to_broadcast([ROWS, 1]))

            # truth columns per 128-chunk: t on partitions (for endgame)
            # tcol[g][:, 0:4] = (tx1, ty1, tx2, ty2)[t],  from strided DMA
            tcols_g = []
            for g in range(2):
                tc4 = pe.tile([ROWS, 4], f32, name=f"tcols{g}")
                nc.sync.dma_start(
                    out=tc4[:],
                    in_=truths4_ext[:, g * ROWS : (g + 1) * ROWS].rearrange(
                        "c t -> t c"
                    ),
                )
                tcols_g.append(tc4)

            # fp16 truth tiles + derived
            TX1h = pe.tile([ROWS, T], f16, name="TX1h")
            TX2h = pe.tile([ROWS, T], f16, name="TX2h")
            TY1h = pe.tile([ROWS, T], f16, name="TY1h")
            TY2h = pe.tile([ROWS, T], f16, name="TY2h")
            nc.vector.tensor_copy(out=TX1h[:], in_=TX1b[:])
            nc.vector.tensor_copy(out=TX2h[:], in_=TX2b[:])
            nc.vector.tensor_copy(out=TY1h[:], in_=TY1b[:])
            nc.vector.tensor_copy(out=TY2h[:], in_=TY2b[:])
            TW32 = pe.tile([ROWS, T], f32, name="TW32")
            TH32 = pe.tile([ROWS, T], f32, name="TH32")
            TA32 = pe.tile([ROWS, T], f32, name="TA32")
            nc.vector.tensor_tensor(out=TW32[:], in0=TX2b[:], in1=TX1b[:], op=OP.subtract)
            nc.vector.tensor_tensor(out=TH32[:], in0=TY2b[:], in1=TY1b[:], op=OP.subtract)
            nc.vector.tensor_tensor(out=TA32[:], in0=TW32[:], in1=TH32[:], op=OP.mult)
            TAh = pe.tile([ROWS, T], f16, name="TAh")
            nc.vector.tensor_copy(out=TAh[:], in_=TA32[:])

            # payload table PAY[t, c] = (10*tcx, 10*tcy, 5*ln tw, 5*ln th)
            # as per-128-chunk columns [t-part, 4] and a DRAM copy for gathers
            pay_dram = dp.tile([T, 4], f32, name="pay_dram")
            PAYg = []
            for g in range(2):
                tc4 = tcols_g[g]
                pay_t = pe.tile([ROWS, 4], f32, name=f"PAY{g}")
                # 10*tcx = 5*(tx1+tx2), 10*tcy = 5*(ty1+ty2)
                nc.vector.tensor_tensor(
                    out=pay_t[:, 0:1], in0=tc4[:, 0:1], in1=tc4[:, 2:3], op=OP.add
                )
                nc.vector.tensor_tensor(
                    out=pay_t[:, 1:2], in0=tc4[:, 1:2], in1=tc4[:, 3:4], op=OP.add
                )
                nc.vector.tensor_scalar(
                    out=pay_t[:, 0:2], in0=pay_t[:, 0:2], scalar1=5.0, scalar2=None,
                    op0=OP.mult,
                )
                twc = wp.tile([ROWS, 2], f32, name=f"twc{g}", tag="twc")
                nc.vector.tensor_tensor(
                    out=twc[:, 0:1], in0=tc4[:, 2:3], in1=tc4[:, 0:1], op=OP.subtract
                )
                nc.vector.tensor_tensor(
                    out=twc[:, 1:2], in0=tc4[:, 3:4], in1=tc4[:, 1:2], op=OP.subtract
                )
                nc.scalar.activation(out=pay_t[:, 2:4], in_=twc[:], func=AF.Ln)
                nc.vector.tensor_scalar(
                    out=pay_t[:, 2:4], in0=pay_t[:, 2:4], scalar1=5.0, scalar2=None,
                    op0=OP.mult,
                )
                nc.sync.dma_start(out=pay_dram[g * ROWS : (g + 1) * ROWS, :], in_=pay_t[:])
                PAYg.append(pay_t)
            # truth area column per chunk: ta = tw*th
            TAc = []
            for g in range(2):
                tc4 = tcols_g[g]
                tac = pe.tile([ROWS, 1], f32, name=f"TAc{g}")
                tw1 = wp.tile([ROWS, 1], f32, name=f"tw1{g}", tag="tw1")
                th1 = wp.tile([ROWS, 1], f32, name=f"th1{g}", tag="th1")
                nc.vector.tensor_tensor(
                    out=tw1[:], in0=tc4[:, 2:3], in1=tc4[:, 0:1], op=OP.subtract
                )
                nc.vector.tensor_tensor(
                    out=th1[:], in0=tc4[:, 3:4], in1=tc4[:, 1:2], op=OP.subtract
                )
                nc.vector.tensor_tensor(out=tac[:], in0=tw1[:], in1=th1[:], op=OP.mult)
                TAc.append(tac)

            # iotas
            it_i = pe.tile([ROWS, T], i32, name="it_i")
            nc.gpsimd.iota(it_i[:], [[1, T]], base=0, channel_multiplier=0)
            IOTA_TF = pe.tile([ROWS, T], f32, name="IOTA_TF")
            nc.vector.tensor_copy(out=IOTA_TF[:], in_=it_i[:])
            rid_i = pe.tile([ROWS, 1], i32, name="rid_i")
            nc.gpsimd.iota(rid_i[:], [[0, 1]], base=0, channel_multiplier=1)
            ridf = pe.tile([ROWS, 1], f32, name="ridf")
            nc.vector.tensor_copy(out=ridf[:], in_=rid_i[:])

            # per-prior derived arrays
            cxv = locs_sb[:, :, 0]
            cyv = locs_sb[:, :, 1]
            wv = params_sb[:, :, 0]
            hv = params_sb[:, :, 1]
            av = params_sb[:, :, 2]

            px1_32 = pe.tile([ROWS, NT], f32, name="px1_32")
            py1_32 = pe.tile([ROWS, NT], f32, name="py1_32")
            pw_32 = pe.tile([ROWS, NT], f32, name="pw_32")
            ph_32 = pe.tile([ROWS, NT], f32, name="ph_32")
            areap = pe.tile([ROWS, NT], f32, name="areap")
            pcx10 = pe.tile([ROWS, NT], f32, name="pcx10")
            pcy10 = pe.tile([ROWS, NT], f32, name="pcy10")
            rpw = pe.tile([ROWS, NT], f32, name="rpw")
            rph = pe.tile([ROWS, NT], f32, name="rph")
            lpw5 = pe.tile([ROWS, NT], f32, name="lpw5")
            lph5 = pe.tile([ROWS, NT], f32, name="lph5")

            halfw = wp.tile([ROWS, NT], f32, name="halfw", tag="halfw")
            halfh = wp.tile([ROWS, NT], f32, name="halfh", tag="halfh")
            nc.vector.tensor_scalar(
                out=halfw[:], in0=wv, scalar1=0.5, scalar2=None, op0=OP.mult
            )
            nc.vector.tensor_scalar(
                out=halfh[:], in0=hv, scalar1=0.5, scalar2=None, op0=OP.mult
            )
            nc.vector.tensor_tensor(out=px1_32[:], in0=cxv, in1=halfw[:], op=OP.subtract)
            nc.vector.tensor_tensor(out=py1_32[:], in0=cyv, in1=halfh[:], op=OP.subtract)
            nc.scalar.copy(out=pw_32[:], in_=wv)
            nc.scalar.copy(out=ph_32[:], in_=hv)
            nc.vector.tensor_tensor(out=areap[:], in0=wv, in1=hv, op=OP.mult)
            nc.vector.tensor_scalar(
                out=pcx10[:], in0=cxv, scalar1=10.0, scalar2=None, op0=OP.mult
            )
            nc.vector.tensor_scalar(
                out=pcy10[:], in0=cyv, scalar1=10.0, scalar2=None, op0=OP.mult
            )
            nc.vector.reciprocal(out=rpw[:], in_=wv)
            nc.vector.reciprocal(out=rph[:], in_=hv)
            rpa = pe.tile([ROWS, NT], f32, name="rpa")
            nc.vector.reciprocal(out=rpa[:], in_=areap[:])
            npy1 = pe.tile([ROWS, NT], f32, name="npy1")
            nc.vector.tensor_scalar(
                out=npy1[:], in0=py1_32[:], scalar1=-1.0, scalar2=None, op0=OP.mult
            )
            # pyh = py1 + ph (for RMy = relu(pyh - ty2))
            pyh = pe.tile([ROWS, NT], f32, name="pyh")
            nc.vector.tensor_tensor(out=pyh[:], in0=py1_32[:], in1=hv, op=OP.add)
            nc.scalar.activation(out=lpw5[:], in_=wv, func=AF.Ln)
            nc.vector.tensor_scalar(
                out=lpw5[:], in0=lpw5[:], scalar1=5.0, scalar2=None, op0=OP.mult
            )
            nc.scalar.activation(out=lph5[:], in_=hv, func=AF.Ln)
            nc.vector.tensor_scalar(
                out=lph5[:], in0=lph5[:], scalar1=5.0, scalar2=None, op0=OP.mult
            )

            # packed prior arrays (bf16) for the endgame one-hot gathers
            bf16 = mybir.dt.bfloat16
            parr = pe.tile([ROWS, NT, 5], bf16, name="parr")
            nc.vector.tensor_copy(out=parr[:, :, 0], in_=px1_32[:])
            nc.vector.tensor_copy(out=parr[:, :, 1], in_=pw_32[:])
            nc.vector.tensor_copy(out=parr[:, :, 2], in_=py1_32[:])
            nc.vector.tensor_copy(out=parr[:, :, 3], in_=ph_32[:])
            nc.vector.tensor_copy(out=parr[:, :, 4], in_=areap[:])

            # ---------------- Phase 1: main IoU loop ----------------
            # Per j (128 priors x 256 truths):
            #   Act : R1 = 1/(ta*rpa_j + 1)            [reciprocal table]
            #   DVE : Mx = (tx2 - px1) min pw          [ts @4x]
            #         Bx = (tx1 - px1) max 0           [ts @4x]
            #         My = (ty2 - py1) min ph          [ts @4x]
            #         By = (ty1 - py1) max 0           [ts @4x]
            #   Pool: wxu = Mx - Bx, wyu = My - By, wrs = wyu * R1
            #   DVE : f = relu(wxu)*relu(wrs)*rpa_j  (+ max-accum -> btoh)
            #         max_index -> bti8
            #   Pool: runmax over j pairs
            # f == inter/(ta+pa) exactly as before since
            #   rpa/(ta*rpa+1) == 1/(ta+pa).
            btoh = pe.tile([ROWS, NT], f16, name="btoh")
            bti8 = pe.tile([ROWS, NT, 8], u32, name="bti8")
            pay_all = pe.tile([ROWS, NT, 4], f32, name="pay_all")
            runmax2 = pe.tile([ROWS, 2, T], f16, name="runmax2")
            nc.vector.memset(runmax2[:], 0.0)

            fjd = None
            for j in range(NT):
                jj = slice(j, j + 1)
                # Act engine: R1 = 1/(ta*rpa+1); By = relu(ty1-py1);
                #             RMy = relu(py1+ph-ty2)
                rS = wp.tile([ROWS, T], f16, name="rS", tag="rS")
                _raw_activation(
                    nc, rS[:], TAh[:], AF.Reciprocal, 1.0, rpa[:, jj]
                )
                By = wp.tile([ROWS, T], f16, name="By", tag="By")
                nc.scalar.activation(
                    out=By[:], in_=TY1h[:], func=AF.Relu, bias=npy1[:, jj], scale=1.0
                )
                RMy = wp.tile([ROWS, T], f16, name="RMy", tag="RMy")
                nc.scalar.activation(
                    out=RMy[:], in_=TY2h[:], func=AF.Relu, bias=pyh[:, jj], scale=-1.0
                )

                # DVE: wr = x-overlap (custom); ysum = RMy+By; wrs = wr*rS
                # f = relu(wrs) * relu(ph - ysum) * rpa  (= inter/(ta+pa))
                wr = wp.tile([ROWS, T], f16, name="wr", tag="wr")
                nc.vector._custom_dve(
                    OV_OP,
                    out=wr[:],
                    in0=TX2h[:],
                    in1=TX1h[:],
                    s0=px1_32[:, jj],
                    s1=params_sb[:, j, 0:1],
                )
                ysum = wp.tile([ROWS, T], f16, name="ysum", tag="ysum")
                nc.vector.tensor_tensor(out=ysum[:], in0=RMy[:], in1=By[:], op=OP.add)
                wrs = wp.tile([ROWS, T], f16, name="wrs", tag="wrs")
                nc.vector.tensor_tensor(out=wrs[:], in0=wr[:], in1=rS[:], op=OP.mult)

                if j % 2 == 0:
                    fjd = wp.tile([ROWS, 2, T], f16, name="fjd", tag="fjd")
                fj = fjd[:, j % 2, :]
                nc.vector._custom_dve(
                    FMR3_OP,
                    out=fj,
                    in0=wrs[:],
                    in1=ysum[:],
                    s0=rpa[:, jj],
                    s1=params_sb[:, j, 1:2],
                    imm2=0.0,
                    accum_out=btoh[:, jj],
                )
                nc.vector.max_index(
                    bti8[:, j, :], btoh[:, jj].to_broadcast([ROWS, 8]), fj
                )
                if j % 2 == 1:
                    nc.vector.tensor_tensor(
                        out=runmax2[:],
                        in0=runmax2[:],
                        in1=fjd[:],
                        op=OP.max,
                    )
                nc.gpsimd.indirect_dma_start(
                    out=pay_all[:, j, :],
                    out_offset=None,
                    in_=pay_dram[:],
                    in_offset=bass.IndirectOffsetOnAxis(ap=bti8[:, j, 0:1], axis=0),
                )

            # ---------------- Phase 2a: local best-prior + packed AllReduce ---
            runmax = pe.tile([ROWS, T], f32, name="runmax")
            rmh = wp.tile([ROWS, T], f16, name="rmh", tag="rmh")
            nc.vector.tensor_tensor(
                out=rmh[:], in0=runmax2[:, 0, :], in1=runmax2[:, 1, :], op=OP.max
            )
            nc.vector.tensor_copy(out=runmax[:], in_=rmh[:])

            B4s = []
            jstars = []
            pgs = []
            vqs = []
            for g in range(2):
                gsl = slice(g * ROWS, (g + 1) * ROWS)
                psT = pp.tile([ROWS, ROWS], f32, name=f"psT{g}", tag="ps")
                nc.tensor.transpose(out=psT[:], in_=runmax[:, gsl], identity=ident[:])
                rmT = pe.tile([ROWS, ROWS], f32, name=f"rmT{g}")
                nc.scalar.copy(out=rmT[:], in_=psT[:])
                # per-t max over q (on this core)
                m8 = pe.tile([ROWS, 8], f32, name=f"m8_{g}")
                nc.vector.max(m8[:], rmT[:])
                i8 = pe.tile([ROWS, 8], u32, name=f"i8_{g}")
                nc.vector.max_index(i8[:], m8[:], rmT[:])
                qstar = pe.tile([ROWS, 1], f32, name=f"qstar{g}")
                nc.vector.tensor_copy(out=qstar[:], in_=i8[:, 0:1])

                # build E[q, t] = (q == q*[t]) via transpose of broadcast
                qb_ps = pp.tile([ROWS, ROWS], f32, name=f"qb_ps{g}", tag="ps")
                nc.tensor.transpose(
                    out=qb_ps[:],
                    in_=qstar[:, 0:1].to_broadcast([ROWS, ROWS]),
                    identity=ident[:],
                )
                qbT = wp.tile([ROWS, ROWS], f32, name=f"qbT{g}", tag="qbT")
                nc.scalar.copy(out=qbT[:], in_=qb_ps[:])
                E = wp.tile([ROWS, ROWS], bf16, name=f"E{g}", tag="E")
                nc.vector.tensor_scalar(
                    out=E[:], in0=qbT[:], scalar1=ridf[:], scalar2=None, op0=OP.is_equal
                )

                # gather prior rows q*[t] via 2 packed one-hot bf16 matmuls
                B4_ps = pp.tile([ROWS, ROWS, 4], f32, name=f"B4_ps{g}", tag="ps")
                nc.tensor.matmul(
                    out=B4_ps[:],
                    lhsT=E[:],
                    rhs=parr[:, :, 0:4],
                    start=True,
                    stop=True,
                )
                B4 = wp.tile([ROWS, ROWS, 4], f32, name=f"B4{g}", tag="B4")
                nc.scalar.copy(out=B4[:], in_=B4_ps[:])
                B1_ps = pp.tile([ROWS, ROWS], f32, name=f"B1_ps{g}", tag="ps")
                nc.tensor.matmul(
                    out=B1_ps[:],
                    lhsT=E[:],
                    rhs=parr[:, :, 4],
                    start=True,
                    stop=True,
                )
                B_ap = wp.tile([ROWS, ROWS], f32, name=f"B_ap{g}", tag="B_ap")
                nc.scalar.copy(out=B_ap[:], in_=B1_ps[:])
                B_px1 = B4[:, :, 0]
                B_pw = B4[:, :, 1]
                B_py1 = B4[:, :, 2]
                B_ph = B4[:, :, 3]

                tc4 = tcols_g[g]
                wr_re = wp.tile([ROWS, ROWS], f32, name=f"wr_re{g}", tag="wr_re")
                nc.vector._custom_dve(
                    OVF_OP,
                    out=wr_re[:],
                    in0=B_px1,
                    in1=B_pw,
                    s0=tc4[:, 2:3],
                    s1=tc4[:, 0:1],
                )
                hr_re = wp.tile([ROWS, ROWS], f32, name=f"hr_re{g}", tag="hr_re")
                nc.vector._custom_dve(
                    OVF_OP,
                    out=hr_re[:],
                    in0=B_py1,
                    in1=B_ph,
                    s0=tc4[:, 3:4],
                    s1=tc4[:, 1:2],
                )
                S_re = wp.tile([ROWS, ROWS], f32, name=f"S_re{g}", tag="S_re")
                nc.vector.tensor_scalar(
                    out=S_re[:], in0=B_ap[:], scalar1=TAc[g][:], scalar2=None, op0=OP.add
                )
                lS_re = wp.tile([ROWS, ROWS], f32, name=f"lS_re{g}", tag="lS_re")
                nc.scalar.activation(out=lS_re[:], in_=S_re[:], func=AF.Ln)
                rS_re = wp.tile([ROWS, ROWS], f32, name=f"rS_re{g}", tag="rS_re")
                nc.scalar.activation(out=rS_re[:], in_=lS_re[:], func=AF.Exp, scale=-1.0)
                f_re = wp.tile([ROWS, ROWS], f32, name=f"f_re{g}", tag="f_re")
                nc.vector.tensor_tensor(
                    out=f_re[:], in0=wr_re[:], in1=rS_re[:], op=OP.mult
                )
                nc.vector.tensor_tensor(
                    out=f_re[:], in0=f_re[:], in1=hr_re[:], op=OP.mult
                )
                mre8 = wp.tile([ROWS, 8], f32, name=f"mre8{g}", tag="mre8")
                nc.vector.max(mre8[:], f_re[:])
                jre8 = wp.tile([ROWS, 8], u32, name=f"jre8{g}", tag="jre8")
                nc.vector.max_index(jre8[:], mre8[:], f_re[:])
                jstar = pe.tile([ROWS, 1], f32, name=f"jstar{g}")
                nc.vector.tensor_copy(out=jstar[:], in_=jre8[:, 0:1])
                B4s.append(B4)
                jstars.append(jstar)

                # local candidate index (within this core) and quantized value;
                # the final pack (with the candidate's xf bit) is assembled
                # after phase 3 stages st3 to DRAM.
                pg = pe.tile([ROWS, 1], f32, name=f"pg{g}")
                nc.vector.tensor_scalar(
                    out=pg[:], in0=qstar[:], scalar1=float(NT), scalar2=None, op0=OP.mult
                )
                nc.vector.tensor_tensor(out=pg[:], in0=pg[:], in1=jstar[:], op=OP.add)
                vqf = wp.tile([ROWS, 1], f32, name=f"vqf{g}", tag="vqf")
                nc.vector.tensor_scalar(
                    out=vqf[:], in0=m8[:, 0:1], scalar1=128.0, scalar2=None, op0=OP.mult
                )
                vqu = wp.tile([ROWS, 1], u32, name=f"vqu{g}", tag="vqu")
                nc.vector.tensor_copy(out=vqu[:], in_=vqf[:])
                vqf2 = pe.tile([ROWS, 1], f32, name=f"vqf2{g}")
                nc.vector.tensor_copy(out=vqf2[:], in_=vqu[:])
                pgs.append(pg)
                vqs.append(vqf2)

            # candidate prior-row data selection + recomputed l1n (as before)
            from concourse.dve_ops import TENSOR_TENSOR_REDUCE as TTR_OP

            gsel = []
            l1ns = []
            for g in range(2):
                ohj = wp.tile([ROWS, ROWS], f32, name=f"ohj{g}", tag="ohj")
                nc.vector.tensor_scalar(
                    out=ohj[:],
                    in0=IOTA_TF[:, 0:ROWS],
                    scalar1=jstars[g][:],
                    scalar2=None,
                    op0=OP.is_equal,
                )
                cols = pe.tile([ROWS, 4], f32, name=f"gselc{g}")
                for c in range(4):
                    trash = wp.tile([ROWS, ROWS], f32, name=f"tsel{g}_{c}", tag="tsel")
                    nc.vector._custom_dve(
                        TTR_OP,
                        out=trash[:],
                        in0=B4s[g][:, :, c],
                        in1=ohj[:],
                        s0=0.0,
                        s1=1.0,
                        accum_out=cols[:, c : c + 1],
                    )
                gsel.append(cols)
                tcols = [PAYg[g][:, c : c + 1] for c in range(4)]
                gw = cols[:, 1:2]
                gh = cols[:, 3:4]
                gcx = pe.tile([ROWS, 1], f32, name=f"gcx{g}")
                nc.vector.tensor_scalar(
                    out=gcx[:], in0=gw, scalar1=0.5, scalar2=None, op0=OP.mult
                )
                nc.vector.tensor_tensor(
                    out=gcx[:], in0=gcx[:], in1=cols[:, 0:1], op=OP.add
                )
                gcy = pe.tile([ROWS, 1], f32, name=f"gcy{g}")
                nc.vector.tensor_scalar(
                    out=gcy[:], in0=gh, scalar1=0.5, scalar2=None, op0=OP.mult
                )
                nc.vector.tensor_tensor(
                    out=gcy[:], in0=gcy[:], in1=cols[:, 2:3], op=OP.add
                )
                rgw = pe.tile([ROWS, 1], f32, name=f"rgw{g}")
                nc.vector.reciprocal(out=rgw[:], in_=gw)
                rgh = pe.tile([ROWS, 1], f32, name=f"rgh{g}")
                nc.vector.reciprocal(out=rgh[:], in_=gh)
                lgw5 = pe.tile([ROWS, 1], f32, name=f"lgw5{g}")
                nc.scalar.activation(out=lgw5[:], in_=gw, func=AF.Ln)
                nc.vector.tensor_scalar(
                    out=lgw5[:], in0=lgw5[:], scalar1=5.0, scalar2=None, op0=OP.mult
                )
                lgh5 = pe.tile([ROWS, 1], f32, name=f"lgh5{g}")
                nc.scalar.activation(out=lgh5[:], in_=gh, func=AF.Ln)
                nc.vector.tensor_scalar(
                    out=lgh5[:], in0=lgh5[:], scalar1=5.0, scalar2=None, op0=OP.mult
                )

                encs = []
                for cc, (tcol, gcen, rg_) in enumerate(
                    ((tcols[0], gcx, rgw), (tcols[1], gcy, rgh))
                ):
                    gx10 = wp.tile([ROWS, 1], f32, name=f"gx10_{g}_{cc}", tag="gx10")
                    nc.vector.tensor_scalar(
                        out=gx10[:], in0=gcen, scalar1=10.0, scalar2=None, op0=OP.mult
                    )
                    en = pe.tile([ROWS, 1], f32, name=f"en{g}_{cc}")
                    nc.vector.tensor_tensor(
                        out=en[:], in0=tcol, in1=gx10[:], op=OP.subtract
                    )
                    nc.vector.tensor_tensor(out=en[:], in0=en[:], in1=rg_[:], op=OP.mult)
                    encs.append(en)
                for cc, (tcol, lg_) in enumerate(((tcols[2], lgw5), (tcols[3], lgh5))):
                    en = pe.tile([ROWS, 1], f32, name=f"en{g}_{cc + 2}")
                    nc.vector.tensor_tensor(
                        out=en[:], in0=tcol, in1=lg_[:], op=OP.subtract
                    )
                    encs.append(en)

                l1n = pe.tile([ROWS, 1], f32, name=f"l1n{g}")
                nc.vector.memset(l1n[:], 0.0)
                for cc, en in enumerate(encs):
                    slv = wp.tile([ROWS, 1], f32, name=f"slv{g}_{cc}", tag="slv")
                    nc.vector._custom_dve(SL1_OP, out=slv[:], in0=en[:], s0=1.0, s1=0.0)
                    nc.vector.tensor_tensor(out=l1n[:], in0=l1n[:], in1=slv[:], op=OP.add)

                l1ns.append(l1n)

            # ---------------- Phase 3: per-prior epilogue ----------------
            xf = pe.tile([ROWS, NT], f32, name="xf")
            nc.vector.tensor_scalar(
                out=xf[:], in0=btoh[:], scalar1=TH_F, scalar2=None, op0=OP.is_gt
            )
            ptcx = pay_all[:, :, 0]
            ptcy = pay_all[:, :, 1]
            ptlw = pay_all[:, :, 2]
            ptlh = pay_all[:, :, 3]
            e_tiles = []
            for idx, (pay, cen, rp) in enumerate(
                ((ptcx, pcx10, rpw), (ptcy, pcy10, rph))
            ):
                e = wp.tile([ROWS, NT], f32, name=f"e{idx}", tag=f"e{idx}")
                nc.vector.tensor_tensor(out=e[:], in0=pay, in1=cen[:], op=OP.subtract)
                nc.vector.tensor_tensor(out=e[:], in0=e[:], in1=rp[:], op=OP.mult)
                e_tiles.append(e)
            for idx, (pay, lp) in enumerate(((ptlw, lpw5), (ptlh, lph5))):
                e = wp.tile([ROWS, NT], f32, name=f"e{idx + 2}", tag=f"e{idx + 2}")
                nc.vector.tensor_tensor(out=e[:], in0=pay, in1=lp[:], op=OP.subtract)
                e_tiles.append(e)

            l1u = pe.tile([ROWS, NT], f32, name="l1u")
            sl_prev = None
            for idx, e in enumerate(e_tiles):
                sl = wp.tile([ROWS, NT], f32, name=f"sl{idx}", tag=f"sl{idx}")
                nc.vector._custom_dve(SL1_OP, out=sl[:], in0=e[:], s0=1.0, s1=0.0)
                if idx == 1:
                    nc.vector.tensor_tensor(
                        out=l1u[:], in0=sl_prev[:], in1=sl[:], op=OP.add
                    )
                elif idx > 1:
                    nc.vector.tensor_tensor(out=l1u[:], in0=l1u[:], in1=sl[:], op=OP.add)
                sl_prev = sl

            term = pe.tile([ROWS, NT], f32, name="term")
            nc.vector.tensor_tensor(out=term[:], in0=s_sb[:], in1=xf[:], op=OP.mult)
            nc.vector.tensor_tensor(out=term[:], in0=term[:], in1=l1u[:], op=OP.mult)
            pack3 = pe.tile([ROWS, 3], f32, name="pack3")
            nc.vector.tensor_reduce(
                out=pack3[:, 0:1], in_=term[:], axis=mybir.AxisListType.X, op=OP.add
            )
            nc.vector.tensor_reduce(
                out=pack3[:, 1:2], in_=xf[:], axis=mybir.AxisListType.X, op=OP.add
            )
            nc.vector.tensor_reduce(
                out=pack3[:, 2:3], in_=s_sb[:], axis=mybir.AxisListType.X, op=OP.add
            )
            sums_ps = pp.tile([1, 3], f32, name="sums_ps", tag="ps")
            nc.tensor.matmul(
                out=sums_ps[:], lhsT=ones128[:], rhs=pack3[:], start=True, stop=True
            )
            sums_sb = pe.tile([1, 3], f32, name="sums_sb")
            nc.scalar.copy(out=sums_sb[:], in_=sums_ps[:])

            # stage per-prior arrays to DRAM for phase-4 indirect gathers
            st3 = pe.tile([ROWS, NT, 3], f32, name="st3")
            nc.scalar.copy(out=st3[:, :, 0], in_=l1u[:])
            nc.scalar.copy(out=st3[:, :, 1], in_=xf[:])
            nc.scalar.copy(out=st3[:, :, 2], in_=s_sb[:])
            st3_dram = dp.tile([PL, 3], f32, name="st3_dram")
            nc.sync.dma_start(
                out=st3_dram[:].rearrange("(r j) c -> r j c", r=ROWS), in_=st3[:]
            )

            # ======== merged endgame: one AllGather carries ========
            #   [0:256)   per-core candidate pack  (qval<<18 | xf<<17 | p_global)
            #   [256:512) per-core candidate dnum  (s*(K*l1n - xf*l1u))
            #   [512:515) per-core partial sums    (S1, S2, S3)
            # every core then unpacks winners, dedups, and computes the loss.
            ag_in = dp.tile([1, 516], f32, name="ag_in")
            for g in range(2):
                phu = pe.tile([ROWS, 1], u32, name=f"phu{g}")
                nc.vector.tensor_copy(out=phu[:], in_=pgs[g][:])
                gat3 = pe.tile([ROWS, 3], f32, name=f"gat3{g}")
                nc.gpsimd.indirect_dma_start(
                    out=gat3[:],
                    out_offset=None,
                    in_=st3_dram[:],
                    in_offset=bass.IndirectOffsetOnAxis(ap=phu[:, 0:1], axis=0),
                )
                l1_at = gat3[:, 0:1]
                xf_at = gat3[:, 1:2]
                s_at = gat3[:, 2:3]

                w1 = wp.tile([ROWS, 1], f32, name=f"w1{g}", tag="w1")
                nc.vector.tensor_scalar(
                    out=w1[:], in0=l1ns[g][:], scalar1=K_VAL, scalar2=None, op0=OP.mult
                )
                w2 = wp.tile([ROWS, 1], f32, name=f"w2{g}", tag="w2")
                nc.vector.tensor_tensor(out=w2[:], in0=xf_at, in1=l1_at, op=OP.mult)
                nc.vector.tensor_tensor(out=w1[:], in0=w1[:], in1=w2[:], op=OP.subtract)
                dn_c = pe.tile([ROWS, 1], f32, name=f"dn_c{g}")
                nc.vector.tensor_tensor(out=dn_c[:], in0=w1[:], in1=s_at, op=OP.mult)

                pk = pe.tile([ROWS, 1], f32, name=f"pk{g}")
                nc.vector.tensor_scalar(
                    out=pk[:],
                    in0=vqs[g][:],
                    scalar1=63.0,
                    scalar2=262144.0,
                    op0=OP.min,
                    op1=OP.mult,
                )
                pxf = wp.tile([ROWS, 1], f32, name=f"pxf{g}", tag="pxf")
                nc.vector.tensor_scalar(
                    out=pxf[:], in0=xf_at, scalar1=131072.0, scalar2=None, op0=OP.mult
                )
                nc.vector.tensor_tensor(out=pk[:], in0=pk[:], in1=pxf[:], op=OP.add)
                nc.vector.tensor_tensor(out=pk[:], in0=pk[:], in1=pgs[g][:], op=OP.add)
                nc.vector.tensor_tensor(out=pk[:], in0=pk[:], in1=POFFb[:], op=OP.add)
                nc.sync.dma_start(
                    out=ag_in[0, g * ROWS : (g + 1) * ROWS], in_=pk[:, 0:1]
                )
                nc.sync.dma_start(
                    out=ag_in[0, 256 + g * ROWS : 256 + (g + 1) * ROWS],
                    in_=dn_c[:, 0:1],
                )
            pad4 = pe.tile([1, 4], f32, name="pad4")
            nc.vector.memset(pad4[:], 0.0)
            nc.vector.tensor_copy(out=pad4[:, 0:3], in_=sums_sb[:])
            nc.sync.dma_start(out=ag_in[0, 512:516], in_=pad4[:])

            ag_out = dp.tile([8, 516], f32, name="ag_out", addr_space="Shared")
            nc.gpsimd.collective_compute(
                "AllGather",
                mybir.AluOpType.bypass,
                ins=[ag_in[:]],
                outs=[ag_out[:]],
                replica_groups=RG,
            )

            # ---- post-collective: winners, dedup, corrections, loss ----
            p_star = []
            xfbs = []
            dnws = []
            for g in range(2):
                gsl = slice(g * ROWS, (g + 1) * ROWS)
                pk8 = pe.tile([ROWS, 8], f32, name=f"pk8_{g}")
                nc.sync.dma_start(
                    out=pk8[:], in_=ag_out[:, gsl].rearrange("l t -> t l")
                )
                win = pe.tile([ROWS, 1], f32, name=f"win{g}")
                nc.vector.tensor_reduce(
                    out=win[:], in_=pk8[:], axis=mybir.AxisListType.X, op=OP.max
                )
                # unpack: qval = floor(win/2^18); rem = win - qval*2^18
                vq1 = wp.tile([ROWS, 1], f32, name=f"vq1{g}", tag="vq1")
                nc.vector.tensor_scalar(
                    out=vq1[:], in0=win[:], scalar1=1.0 / 262144.0, scalar2=None,
                    op0=OP.mult,
                )
                vq1u = wp.tile([ROWS, 1], u32, name=f"vq1u{g}", tag="vq1u")
                nc.vector.tensor_copy(out=vq1u[:], in_=vq1[:])
                vq1f = wp.tile([ROWS, 1], f32, name=f"vq1f{g}", tag="vq1f")
                nc.vector.tensor_copy(out=vq1f[:], in_=vq1u[:])
                rem = pe.tile([ROWS, 1], f32, name=f"rem{g}")
                nc.vector.tensor_scalar(
                    out=rem[:], in0=vq1f[:], scalar1=-262144.0, scalar2=None,
                    op0=OP.mult,
                )
                nc.vector.tensor_tensor(out=rem[:], in0=rem[:], in1=win[:], op=OP.add)
                # xfb = floor(rem/2^17); p = rem - xfb*2^17
                xb1 = wp.tile([ROWS, 1], f32, name=f"xb1{g}", tag="xb1")
                nc.vector.tensor_scalar(
                    out=xb1[:], in0=rem[:], scalar1=1.0 / 131072.0, scalar2=None,
                    op0=OP.mult,
                )
                xb1u = wp.tile([ROWS, 1], u32, name=f"xb1u{g}", tag="xb1u")
                nc.vector.tensor_copy(out=xb1u[:], in_=xb1[:])
                xfb = pe.tile([ROWS, 1], f32, name=f"xfb{g}")
                nc.vector.tensor_copy(out=xfb[:], in_=xb1u[:])
                ps_col = pe.tile([ROWS, 1], f32, name=f"ps_col{g}")
                nc.vector.tensor_scalar(
                    out=ps_col[:], in0=xfb[:], scalar1=-131072.0, scalar2=None,
                    op0=OP.mult,
                )
                nc.vector.tensor_tensor(
                    out=ps_col[:], in0=ps_col[:], in1=rem[:], op=OP.add
                )
                # lane = floor(p / PL)
                ln1 = wp.tile([ROWS, 1], f32, name=f"ln1{g}", tag="ln1")
                nc.vector.tensor_scalar(
                    out=ln1[:], in0=ps_col[:], scalar1=1.0 / float(PL), scalar2=None,
                    op0=OP.mult,
                )
                ln1u = wp.tile([ROWS, 1], u32, name=f"ln1u{g}", tag="ln1u")
                nc.vector.tensor_copy(out=ln1u[:], in_=ln1[:])
                lanef = pe.tile([ROWS, 1], f32, name=f"lanef{g}")
                nc.vector.tensor_copy(out=lanef[:], in_=ln1u[:])

                d8 = pe.tile([ROWS, 8], f32, name=f"d8_{g}")
                nc.sync.dma_start(
                    out=d8[:],
                    in_=ag_out[:, 256 + g * ROWS : 256 + (g + 1) * ROWS].rearrange(
                        "l t -> t l"
                    ),
                )
                oh8 = wp.tile([ROWS, 8], f32, name=f"oh8{g}", tag="oh8")
                nc.vector.tensor_scalar(
                    out=oh8[:], in0=IOTA_TF[:, 0:8], scalar1=lanef[:], scalar2=None,
                    op0=OP.is_equal,
                )
                dsel = wp.tile([ROWS, 8], f32, name=f"dsel{g}", tag="dsel")
                nc.vector.tensor_tensor(out=dsel[:], in0=oh8[:], in1=d8[:], op=OP.mult)
                dnw = pe.tile([ROWS, 1], f32, name=f"dnw{g}")
                nc.vector.tensor_reduce(
                    out=dnw[:], in_=dsel[:], axis=mybir.AxisListType.X, op=OP.add
                )
                p_star.append(ps_col)
                xfbs.append(xfb)
                dnws.append(dnw)

            # dedup: a prior claimed by several truths keeps only the last t
            PSb = pe.tile([ROWS, T], f32, name="PSb")
            for g in range(2):
                psb_ps = pp.tile([ROWS, ROWS], f32, name=f"psb_ps{g}", tag="ps")
                nc.tensor.transpose(
                    out=psb_ps[:],
                    in_=p_star[g][:, 0:1].to_broadcast([ROWS, ROWS]),
                    identity=ident[:],
                )
                nc.scalar.copy(out=PSb[:, g * ROWS : (g + 1) * ROWS], in_=psb_ps[:])

            keep = []
            for g in range(2):
                eqm = wp.tile([ROWS, T], f32, name=f"eqm{g}", tag="eqm")
                nc.vector.tensor_scalar(
                    out=eqm[:],
                    in0=PSb[:],
                    scalar1=p_star[g][:, 0:1],
                    scalar2=None,
                    op0=OP.is_equal,
                )
                rid_g = pe.tile([ROWS, 1], f32, name=f"rid_g{g}")
                nc.vector.tensor_scalar(
                    out=rid_g[:],
                    in0=ridf[:],
                    scalar1=float(g * ROWS),
                    scalar2=None,
                    op0=OP.add,
                )
                trg = wp.tile([ROWS, T], f32, name=f"trg{g}", tag="trg")
                nc.vector.tensor_scalar(
                    out=trg[:],
                    in0=IOTA_TF[:],
                    scalar1=rid_g[:],
                    scalar2=None,
                    op0=OP.is_gt,
                )
                anyl = pe.tile([ROWS, 1], f32, name=f"anyl{g}")
                trash3 = wp.tile([ROWS, T], f32, name=f"trash3{g}", tag="trash3")
                nc.vector.tensor_tensor(out=trash3[:], in0=eqm[:], in1=trg[:], op=OP.mult)
                nc.vector.tensor_reduce(
                    out=anyl[:], in_=trash3[:], axis=mybir.AxisListType.X, op=OP.max
                )
                kp = pe.tile([ROWS, 1], f32, name=f"keep{g}")
                nc.vector.tensor_scalar(
                    out=kp[:],
                    in0=anyl[:],
                    scalar1=-1.0,
                    scalar2=1.0,
                    op0=OP.mult,
                    op1=OP.add,
                )
                keep.append(kp)

            dn_g = []
            dd_g = []
            for g in range(2):
                dn = pe.tile([ROWS, 1], f32, name=f"dn{g}")
                nc.vector.tensor_tensor(
                    out=dn[:], in0=dnws[g][:], in1=keep[g][:], op=OP.mult
                )
                dd = pe.tile([ROWS, 1], f32, name=f"dd{g}")
                nc.vector.tensor_scalar(
                    out=dd[:],
                    in0=xfbs[g][:],
                    scalar1=-1.0,
                    scalar2=K_VAL,
                    op0=OP.mult,
                    op1=OP.add,
                )
                nc.vector.tensor_tensor(out=dd[:], in0=dd[:], in1=keep[g][:], op=OP.mult)
                dn_g.append(dn)
                dd_g.append(dd)

            pack2 = pe.tile([ROWS, 2], f32, name="pack2")
            nc.vector.tensor_tensor(
                out=pack2[:, 0:1], in0=dn_g[0][:], in1=dn_g[1][:], op=OP.add
            )
            nc.vector.tensor_tensor(
                out=pack2[:, 1:2], in0=dd_g[0][:], in1=dd_g[1][:], op=OP.add
            )
            sums2_ps = pp.tile([1, 2], f32, name="sums2_ps", tag="ps")
            nc.tensor.matmul(
                out=sums2_ps[:], lhsT=ones128[:], rhs=pack2[:], start=True, stop=True
            )
            sums2_sb = pe.tile([1, 2], f32, name="sums2_sb")
            nc.scalar.copy(out=sums2_sb[:], in_=sums2_ps[:])

            # global S1..S3: sum the 8 gathered lanes
            s83 = pe.tile([8, 3], f32, name="s83")
            nc.sync.dma_start(out=s83[:], in_=ag_out[:, 512:515])
            sg_ps = pp.tile([1, 3], f32, name="sg_ps", tag="ps")
            nc.tensor.matmul(
                out=sg_ps[:], lhsT=ones128[0:8, 0:1], rhs=s83[:], start=True, stop=True
            )
            sumsg = pe.tile([1, 3], f32, name="sumsg")
            nc.scalar.copy(out=sumsg[:], in_=sg_ps[:])

            num = pe.tile([1, 1], f32, name="num")
            nc.vector.tensor_tensor(
                out=num[:], in0=sumsg[:, 0:1], in1=sums2_sb[:, 0:1], op=OP.add
            )
            nc.vector.tensor_scalar(
                out=num[:], in0=num[:], scalar1=0.5, scalar2=None, op0=OP.mult
            )
            nc.vector.tensor_tensor(
                out=num[:], in0=num[:], in1=sumsg[:, 2:3], op=OP.add
            )
            den = pe.tile([1, 1], f32, name="den")
            nc.vector.tensor_tensor(
                out=den[:], in0=sumsg[:, 1:2], in1=sums2_sb[:, 1:2], op=OP.add
            )
            rden = pe.tile([1, 1], f32, name="rden")
            nc.vector.reciprocal(out=rden[:], in_=den[:])
            loss = pe.tile([1, 1], f32, name="loss")
            nc.vector.tensor_tensor(out=loss[:], in0=num[:], in1=rden[:], op=OP.mult)
            nc.sync.dma_start(out=out_ext[:], in_=loss[:])

    from concourse import mybir as _mb

    _mb.codegen_inst_isa_subclasses(nc)
    _split_waits(nc)
    return nc


def _split_waits(nc):
    """This toolchain's codegen accepts only one embedded sem-wait per
    instruction; hoist extra waits into standalone EventSemaphore
    instructions on the same engine (same blocking semantics)."""
    import orjson

    import copy as _copy

    d = orjson.loads(nc.to_json_bytes())
    ctr = 0
    for fn in d.get("functions", []):
        for bb in fn.get("blocks", []):
            out = []
            for ins in bb.get("instructions", []):
                if (
                    ins.get("opcode") == "ISA"
                    and ins.get("op_name") == "EVENT_SEMAPHORE_RANGE_CLEAR"
                ):
                    # codegen rejects clear ranges wider than 16 sems; split.
                    first, last = ins["instr"][13], ins["instr"][14]
                    if last - first + 1 > 16:
                        lo = first
                        while lo <= last:
                            hi = min(lo + 15, last)
                            ctr += 1
                            part = _copy.deepcopy(ins)
                            part["name"] = f"{ins['name']}_rc{ctr}"
                            part["instr"] = list(ins["instr"])
                            part["instr"][13] = lo
                            part["instr"][14] = hi
                            if lo != first:
                                part["sync_info"] = {"on_wait": [], "on_update": []}
                            out.append(part)
                            lo = hi + 1
                        continue
                si = ins.get("sync_info")
                ow = (si or {}).get("on_wait") or []
                if si and len(ow) > 1 and "engine" in ins:
                    for w in ow[:-1]:
                        ctr += 1
                        ev = {
                            "engine": ins["engine"],
                            "ins": [],
                            "outs": [],
                            "name": f"antsplit_{ctr}",
                            "opcode": "EventSemaphore",
                            "sync_info": {"on_wait": [w], "on_update": []},
                        }
                        if "debug" in ins:
                            ev["debug"] = ins["debug"]
                        out.append(ev)
                    si["on_wait"] = [ow[-1]]
                out.append(ins)
            bb["instructions"] = out
    blob = orjson.dumps(d)
    nc.to_json_bytes = lambda: blob
    return nc


def kernel(**inputs):
    locs = np.ascontiguousarray(np.asarray(inputs["locs"], dtype=np.float32))
    params = np.ascontiguousarray(np.asarray(inputs["params"], dtype=np.float32))
    truths = np.ascontiguousarray(np.asarray(inputs["truths"], dtype=np.float32))
    truths4 = np.ascontiguousarray(truths.T)

    if "nc" not in _CACHE:
        _CACHE["nc"] = _build()
    nc = _CACHE["nc"]

    in_maps = []
    for c in range(NCORES):
        in_maps.append(
            {
                "locs": locs[c * PL : (c + 1) * PL],
                "params": params[c * PL : (c + 1) * PL],
                "truths4": truths4,
                "poff": np.array([[c * PL]], dtype=np.float32),
            }
        )

    from concourse.bass_utils import run_bass_kernel_spmd

    res = run_bass_kernel_spmd(nc, in_maps, core_ids=list(range(NCORES)))
    out = np.asarray(res.results[0]["out"], dtype=np.float32)
    return out.reshape(())


if __name__ == "__main__":
    sys.path.insert(0, "/root/problem")
    import reference

    inputs = {k: np.asarray(v) for k, v in reference.setup_inputs().items()}
    expected = np.asarray(reference.reference(**inputs))
    actual = kernel(**inputs)
    rel = abs(float(actual) - float(expected)) / max(abs(float(expected)), 1e-12)
    print("expected:", expected, "actual:", actual, "rel_err:", rel)



# revision 3
# speedup vs baseline: 1.0721x; 1.0721x over previous
import sys

sys.path.insert(0, "/opt/trn_rl_repo")

import numpy as np

# Problem constants (hardcoded per spec nn_AdaptivePriorBoxesLoss)
P_TOT = 131072
T = 256
NCORES = 8
PL = P_TOT // NCORES  # 16384 priors per core
ROWS = 128
NT = PL // ROWS  # 128 tiles per core; local prior p = q*NT + j
TH_F = 2.0 / 7.0  # iou > 0.4  <=>  inter/(areaA+areaB) > 2/7
K_VAL = 2.5
BETA = 1.0

_CACHE = {}


# ---------------------------------------------------------------------------
# Custom DVE ops (registered into concourse.dve_ops process-locally)
# ---------------------------------------------------------------------------
def _register_custom_ops():
    from concourse import dve_ops
    from concourse.dve_ops import DveOp
    from concourse.dve_spec import (
        Spec,
        Src0,
        Src1,
        C0,
        C1,
        C2,
        Zero,
        relu,
        minn,
        maxx,
        lower,
        _has_src1,
    )
    from concourse.dve_uop import DveOpSpec
    from operator import add as _add

    def mk(name, spec):
        if name in dve_ops._SUB_OPCODE_FOR_NAME:
            for op in dve_ops.OPS:
                if op.name == name:
                    return op
        row = dve_ops._CUSTOM_DVE_ROW_BASE + len(dve_ops.OPS)
        shas = {}
        for ver in ("v3", "v4"):
            u = lower(spec, ver=ver)
            ds = DveOpSpec(name=name, opcode=row, uops=u, rd1_en=_has_src1(spec))
            shas[ver] = ds.sha(ver)
        op = DveOp(name, spec, subdim=False, uops_sha=shas)
        dve_ops.OPS.append(op)
        dve_ops._SUB_OPCODE_FOR_NAME[name] = row
        dve_ops.CUSTOM_DVE_SPECS[name] = spec
        return op

    # interval overlap: out = relu(min(Src0 - s0, s1) - relu(Src1 - s0))
    #  = relu(min(tx2 - px1, pw) - relu(tx1 - px1))  [per-truth free dim]
    def _ov_ref(in0, in1, s0, s1, imm2):
        a = np.minimum(in0.astype(np.float32) - s0, s1)
        b = np.maximum(in1.astype(np.float32) - s0, 0.0)
        return np.maximum(a - b, 0.0).astype(np.float32)

    ov = mk(
        "OVERLAP_LEN_ANT",
        Spec(body=relu(minn(Src0 - C0, C1) - relu(Src1 - C0)), reference=_ov_ref),
    )

    # flipped roles (tensor = prior arrays, scalars = truth coords):
    # out = relu(min(s0 - Src0, Src1) - relu(s1 - Src0))
    def _ovf_ref(in0, in1, s0, s1, imm2):
        a = np.minimum(s0 - in0.astype(np.float32), in1)
        b = np.maximum(s1 - in0.astype(np.float32), 0.0)
        return np.maximum(a - b, 0.0).astype(np.float32)

    ovf = mk(
        "OVERLAP_FLIP_ANT",
        Spec(body=relu(minn(C0 - Src0, Src1) - relu(C1 - Src0)), reference=_ovf_ref),
    )

    # fused elementwise product + running-max accumulation
    def _fm_ref(in0, in1, s0, s1, imm2):
        b = (in0.astype(np.float32) * in1).astype(np.float32)
        return b, np.maximum(
            b.reshape(b.shape[0], -1).max(axis=-1, keepdims=True),
            np.asarray(s0, np.float32).reshape(-1, 1),
        )

    fm = mk(
        "FMUL_MAX_ANT",
        Spec(body=Src0 * Src1, accum=maxx, accum_init=C0, reference=_fm_ref),
    )

    # smooth-l1 core: a=|x|; out = min(a,s0)*a + relu(a-s0)   (s0=1.0)
    def _sl1_ref(in0, in1, s0, s1, imm2):
        aa = np.abs(in0.astype(np.float32))
        return (np.minimum(aa, s0) * aa + np.maximum(aa - s0, 0.0)).astype(np.float32)

    a_expr = maxx(Src0, Zero - Src0)
    sl1 = mk(
        "SMOOTHL1_ANT",
        Spec(body=minn(a_expr, C0) * a_expr + relu(a_expr - C0), reference=_sl1_ref),
    )

    # fused final product: out = relu(in0) * relu(in1) * s0, with running-max
    # accumulation (seeded from s1) into accum_out
    def _fmr_ref(in0, in1, s0, s1, imm2):
        b = (
            np.maximum(in0.astype(np.float32), 0.0)
            * np.maximum(in1.astype(np.float32), 0.0)
            * np.asarray(s0, np.float32).reshape(-1, 1)
        ).astype(np.float32)
        return b, np.maximum(
            b.reshape(b.shape[0], -1).max(axis=-1, keepdims=True),
            np.asarray(s1, np.float32).reshape(-1, 1),
        )

    fmr = mk(
        "FMULRELU_MAX_ANT",
        Spec(
            body=relu(Src0) * relu(Src1) * C0,
            accum=maxx,
            accum_init=C1,
            reference=_fmr_ref,
        ),
    )

    # f = relu(in0) * relu(s1 - in1) * s0, running-max accum (init imm2)
    def _fmr3_ref(in0, in1, s0, s1, imm2):
        b = (
            np.maximum(in0.astype(np.float32), 0.0)
            * np.maximum(
                np.asarray(s1, np.float32).reshape(-1, 1) - in1.astype(np.float32),
                0.0,
            )
            * np.asarray(s0, np.float32).reshape(-1, 1)
        ).astype(np.float32)
        return b, np.maximum(
            b.reshape(b.shape[0], -1).max(axis=-1, keepdims=True),
            np.asarray(imm2, np.float32),
        )

    fmr3 = mk(
        "FMR3_ANT",
        Spec(
            body=relu(Src0) * relu(C1 - Src1) * C0,
            accum=maxx,
            accum_init=C2,
            reference=_fmr3_ref,
        ),
    )
    return ov, ovf, fm, sl1, fmr, fmr3


def _raw_activation(nc, out, in_, func, bias_imm, scale_ap):
    """nc.scalar.activation minus the Reciprocal guard: out = func(in*scale+bias).
    bias is a float immediate (required for Reciprocal), scale a [P,1] AP."""
    from concourse import mybir

    inputs = [
        nc.scalar.lower_ap(in_),
        mybir.ImmediateValue(dtype=mybir.dt.float32, value=float(bias_imm)),
        nc.scalar.lower_ap(scale_ap),
        mybir.ImmediateValue(dtype=mybir.dt.float32, value=0.0),
    ]
    return nc.scalar.add_instruction(
        mybir.InstActivation(
            name=nc.get_next_instruction_name(),
            func=func,
            ins=inputs,
            outs=[nc.scalar.lower_ap(out)],
        )
    )


def _build():
    from concourse import bass, mybir, tile
    from concourse.masks import make_identity

    OV_OP, OVF_OP, FM_OP, SL1_OP, FMR_OP, FMR3_OP = _register_custom_ops()

    f32 = mybir.dt.float32
    f16 = mybir.dt.float16
    i32 = mybir.dt.int32
    u32 = mybir.dt.uint32
    AF = mybir.ActivationFunctionType
    OP = mybir.AluOpType

    nc = bass.Bass()
    locs_ext = nc.declare_dram_parameter("locs", [PL, 2], f32, isOutput=False)
    params_ext = nc.declare_dram_parameter("params", [PL, 3], f32, isOutput=False)
    truths4_ext = nc.declare_dram_parameter("truths4", [4, T], f32, isOutput=False)
    poff_ext = nc.declare_dram_parameter("poff", [1, 1], f32, isOutput=False)
    out_ext = nc.declare_dram_parameter("out", [1, 1], f32, isOutput=True)

    RG = [list(range(NCORES))]

    with tile.TileContext(nc) as tc:
        with (
            tc.tile_pool(name="persist", bufs=1) as pe,
            tc.tile_pool(name="work", bufs=6) as wp,
            tc.tile_pool(name="psum", bufs=2, space="PSUM") as pp,
            tc.tile_pool(name="psumy", bufs=3, space="PSUM") as ppy,
            tc.tile_pool(name="dram", bufs=1, space="DRAM") as dp,
        ):
            # ---------------- Phase 0: prep ----------------
            locs_sb = pe.tile([ROWS, NT, 2], f32, name="locs_sb")
            params_sb = pe.tile([ROWS, NT, 3], f32, name="params_sb")
            nc.sync.dma_start(
                out=locs_sb[:], in_=locs_ext[:].rearrange("(r j) c -> r j c", r=ROWS)
            )
            nc.sync.dma_start(
                out=params_sb[:],
                in_=params_ext[:].rearrange("(r j) c -> r j c", r=ROWS),
            )

            ones128 = pe.tile([ROWS, 1], f32, name="ones128")
            nc.vector.memset(ones128[:], 1.0)
            ident = pe.tile([ROWS, ROWS], f32, name="ident")
            make_identity(nc, ident[:])
            identh = pe.tile([ROWS, ROWS], f16, name="identh")
            nc.vector.tensor_copy(out=identh[:], in_=ident[:])

            # sigmoid first: keeps the ACT table switches to 2 total
            s_sb = pe.tile([ROWS, NT], f32, name="s_sb")
            nc.scalar.activation(
                out=s_sb[:], in_=params_sb[:, :, 2], func=AF.Sigmoid
            )

            # truth broadcast tiles (f32) via row-broadcast DMA
            TX1b = pe.tile([ROWS, T], f32, name="TX1b")
            TY1b = pe.tile([ROWS, T], f32, name="TY1b")
            TX2b = pe.tile([ROWS, T], f32, name="TX2b")
            TY2b = pe.tile([ROWS, T], f32, name="TY2b")
            for rix, bt in enumerate((TX1b, TY1b, TX2b, TY2b)):
                nc.sync.dma_start(
                    out=bt[:], in_=truths4_ext[rix : rix + 1, :].to_broadcast([ROWS, T])
                )
            POFFb = pe.tile([ROWS, 1], f32, name="POFFb")
            nc.sync.dma_start(out=POFFb[:], in_=poff_ext[:].to_broadcast([ROWS, 1]))

            # truth columns per 128-chunk: t on partitions (for endgame)
            # tcol[g][:, 0:4] = (tx1, ty1, tx2, ty2)[t],  from strided DMA
            tcols_g = []
            for g in range(2):
                tc4 = pe.tile([ROWS, 4], f32, name=f"tcols{g}")
                nc.sync.dma_start(
                    out=tc4[:],
                    in_=truths4_ext[:, g * ROWS : (g + 1) * ROWS].rearrange(
                        "c t -> t c"
                    ),
                )
                tcols_g.append(tc4)

            # fp16 truth tiles + derived
            TX1h = pe.tile([ROWS, T], f16, name="TX1h")
            TX2h = pe.tile([ROWS, T], f16, name="TX2h")
            TY1h = pe.tile([ROWS, T], f16, name="TY1h")
            TY2h = pe.tile([ROWS, T], f16, name="TY2h")
            nc.vector.tensor_copy(out=TX1h[:], in_=TX1b[:])
            nc.vector.tensor_copy(out=TX2h[:], in_=TX2b[:])
            nc.vector.tensor_copy(out=TY1h[:], in_=TY1b[:])
            nc.vector.tensor_copy(out=TY2h[:], in_=TY2b[:])
            TW32 = pe.tile([ROWS, T], f32, name="TW32")
            TH32 = pe.tile([ROWS, T], f32, name="TH32")
            TA32 = pe.tile([ROWS, T], f32, name="TA32")
            nc.vector.tensor_tensor(out=TW32[:], in0=TX2b[:], in1=TX1b[:], op=OP.subtract)
            nc.vector.tensor_tensor(out=TH32[:], in0=TY2b[:], in1=TY1b[:], op=OP.subtract)
            nc.vector.tensor_tensor(out=TA32[:], in0=TW32[:], in1=TH32[:], op=OP.mult)
            TAh = pe.tile([ROWS, T], f16, name="TAh")
            nc.vector.tensor_copy(out=TAh[:], in_=TA32[:])

            # payload table PAY[t, c] = (10*tcx, 10*tcy, 5*ln tw, 5*ln th)
            # as per-128-chunk columns [t-part, 4] and a DRAM copy for gathers
            pay_dram = dp.tile([T, 4], f32, name="pay_dram")
            PAYg = []
            for g in range(2):
                tc4 = tcols_g[g]
                pay_t = pe.tile([ROWS, 4], f32, name=f"PAY{g}")
                # 10*tcx = 5*(tx1+tx2), 10*tcy = 5*(ty1+ty2)
                nc.vector.tensor_tensor(
                    out=pay_t[:, 0:1], in0=tc4[:, 0:1], in1=tc4[:, 2:3], op=OP.add
                )
                nc.vector.tensor_tensor(
                    out=pay_t[:, 1:2], in0=tc4[:, 1:2], in1=tc4[:, 3:4], op=OP.add
                )
                nc.vector.tensor_scalar(
                    out=pay_t[:, 0:2], in0=pay_t[:, 0:2], scalar1=5.0, scalar2=None,
                    op0=OP.mult,
                )
                twc = wp.tile([ROWS, 2], f32, name=f"twc{g}", tag="twc")
                nc.vector.tensor_tensor(
                    out=twc[:, 0:1], in0=tc4[:, 2:3], in1=tc4[:, 0:1], op=OP.subtract
                )
                nc.vector.tensor_tensor(
                    out=twc[:, 1:2], in0=tc4[:, 3:4], in1=tc4[:, 1:2], op=OP.subtract
                )
                nc.scalar.activation(out=pay_t[:, 2:4], in_=twc[:], func=AF.Ln)
                nc.vector.tensor_scalar(
                    out=pay_t[:, 2:4], in0=pay_t[:, 2:4], scalar1=5.0, scalar2=None,
                    op0=OP.mult,
                )
                nc.sync.dma_start(out=pay_dram[g * ROWS : (g + 1) * ROWS, :], in_=pay_t[:])
                PAYg.append(pay_t)
            # truth area column per chunk: ta = tw*th
            TAc = []
            for g in range(2):
                tc4 = tcols_g[g]
                tac = pe.tile([ROWS, 1], f32, name=f"TAc{g}")
                tw1 = wp.tile([ROWS, 1], f32, name=f"tw1{g}", tag="tw1")
                th1 = wp.tile([ROWS, 1], f32, name=f"th1{g}", tag="th1")
                nc.vector.tensor_tensor(
                    out=tw1[:], in0=tc4[:, 2:3], in1=tc4[:, 0:1], op=OP.subtract
                )
                nc.vector.tensor_tensor(
                    out=th1[:], in0=tc4[:, 3:4], in1=tc4[:, 1:2], op=OP.subtract
                )
                nc.vector.tensor_tensor(out=tac[:], in0=tw1[:], in1=th1[:], op=OP.mult)
                TAc.append(tac)

            lthr = pe.tile([ROWS, 9], i32, name="lthr")
            nc.gpsimd.iota(lthr[:], [[1, 9]], base=0, channel_multiplier=0)
            lthrf = pe.tile([ROWS, 9], f32, name="lthrf")
            nc.vector.tensor_copy(out=lthrf[:], in_=lthr[:])
            nc.vector.tensor_scalar(
                out=lthrf[:], in0=lthrf[:], scalar1=float(PL), scalar2=None,
                op0=OP.mult,
            )

            # iotas
            it_i = pe.tile([ROWS, T], i32, name="it_i")
            nc.gpsimd.iota(it_i[:], [[1, T]], base=0, channel_multiplier=0)
            IOTA_TF = pe.tile([ROWS, T], f32, name="IOTA_TF")
            nc.vector.tensor_copy(out=IOTA_TF[:], in_=it_i[:])
            rid_i = pe.tile([ROWS, 1], i32, name="rid_i")
            nc.gpsimd.iota(rid_i[:], [[0, 1]], base=0, channel_multiplier=1)
            ridf = pe.tile([ROWS, 1], f32, name="ridf")
            nc.vector.tensor_copy(out=ridf[:], in_=rid_i[:])

            # per-prior derived arrays
            cxv = locs_sb[:, :, 0]
            cyv = locs_sb[:, :, 1]
            wv = params_sb[:, :, 0]
            hv = params_sb[:, :, 1]
            av = params_sb[:, :, 2]

            px1_32 = pe.tile([ROWS, NT], f32, name="px1_32")
            py1_32 = pe.tile([ROWS, NT], f32, name="py1_32")
            pw_32 = pe.tile([ROWS, NT], f32, name="pw_32")
            ph_32 = pe.tile([ROWS, NT], f32, name="ph_32")
            areap = pe.tile([ROWS, NT], f32, name="areap")
            pcx10 = pe.tile([ROWS, NT], f32, name="pcx10")
            pcy10 = pe.tile([ROWS, NT], f32, name="pcy10")
            rpw = pe.tile([ROWS, NT], f32, name="rpw")
            rph = pe.tile([ROWS, NT], f32, name="rph")
            lpw5 = pe.tile([ROWS, NT], f32, name="lpw5")
            lph5 = pe.tile([ROWS, NT], f32, name="lph5")

            halfw = wp.tile([ROWS, NT], f32, name="halfw", tag="halfw")
            halfh = wp.tile([ROWS, NT], f32, name="halfh", tag="halfh")
            nc.vector.tensor_scalar(
                out=halfw[:], in0=wv, scalar1=0.5, scalar2=None, op0=OP.mult
            )
            nc.vector.tensor_scalar(
                out=halfh[:], in0=hv, scalar1=0.5, scalar2=None, op0=OP.mult
            )
            nc.vector.tensor_tensor(out=px1_32[:], in0=cxv, in1=halfw[:], op=OP.subtract)
            nc.vector.tensor_tensor(out=py1_32[:], in0=cyv, in1=halfh[:], op=OP.subtract)
            nc.scalar.copy(out=pw_32[:], in_=wv)
            nc.scalar.copy(out=ph_32[:], in_=hv)
            nc.vector.tensor_tensor(out=areap[:], in0=wv, in1=hv, op=OP.mult)
            nc.vector.tensor_scalar(
                out=pcx10[:], in0=cxv, scalar1=10.0, scalar2=None, op0=OP.mult
            )
            nc.vector.tensor_scalar(
                out=pcy10[:], in0=cyv, scalar1=10.0, scalar2=None, op0=OP.mult
            )
            nc.vector.reciprocal(out=rpw[:], in_=wv)
            nc.vector.reciprocal(out=rph[:], in_=hv)
            rpa = pe.tile([ROWS, NT], f32, name="rpa")
            nc.vector.reciprocal(out=rpa[:], in_=areap[:])
            npy1 = pe.tile([ROWS, NT], f32, name="npy1")
            nc.vector.tensor_scalar(
                out=npy1[:], in0=py1_32[:], scalar1=-1.0, scalar2=None, op0=OP.mult
            )
            # pyh = py1 + ph (for RMy = relu(pyh - ty2))
            pyh = pe.tile([ROWS, NT], f32, name="pyh")
            nc.vector.tensor_tensor(out=pyh[:], in0=py1_32[:], in1=hv, op=OP.add)
            nc.scalar.activation(out=lpw5[:], in_=wv, func=AF.Ln)
            nc.vector.tensor_scalar(
                out=lpw5[:], in0=lpw5[:], scalar1=5.0, scalar2=None, op0=OP.mult
            )
            nc.scalar.activation(out=lph5[:], in_=hv, func=AF.Ln)
            nc.vector.tensor_scalar(
                out=lph5[:], in0=lph5[:], scalar1=5.0, scalar2=None, op0=OP.mult
            )

            # packed prior arrays (bf16) for the endgame one-hot gathers
            bf16 = mybir.dt.bfloat16
            parr = pe.tile([ROWS, NT, 5], bf16, name="parr")
            nc.vector.tensor_copy(out=parr[:, :, 0], in_=px1_32[:])
            nc.vector.tensor_copy(out=parr[:, :, 1], in_=pw_32[:])
            nc.vector.tensor_copy(out=parr[:, :, 2], in_=py1_32[:])
            nc.vector.tensor_copy(out=parr[:, :, 3], in_=ph_32[:])
            nc.vector.tensor_copy(out=parr[:, :, 4], in_=areap[:])

            # ---------------- Phase 1: main IoU loop ----------------
            # Per j (128 priors x 256 truths):
            #   Act : R1 = 1/(ta*rpa_j + 1)            [reciprocal table]
            #   DVE : Mx = (tx2 - px1) min pw          [ts @4x]
            #         Bx = (tx1 - px1) max 0           [ts @4x]
            #         My = (ty2 - py1) min ph          [ts @4x]
            #         By = (ty1 - py1) max 0           [ts @4x]
            #   Pool: wxu = Mx - Bx, wyu = My - By, wrs = wyu * R1
            #   DVE : f = relu(wxu)*relu(wrs)*rpa_j  (+ max-accum -> btoh)
            #         max_index -> bti8
            #   Pool: runmax over j pairs
            # f == inter/(ta+pa) exactly as before since
            #   rpa/(ta*rpa+1) == 1/(ta+pa).
            btoh = pe.tile([ROWS, NT], f16, name="btoh")
            bti8 = pe.tile([ROWS, NT, 8], u32, name="bti8")
            pay_all = pe.tile([ROWS, NT, 4], f32, name="pay_all")
            runmax4 = pe.tile([ROWS, 4, T], f16, name="runmax4")
            nc.vector.memset(runmax4[:], 0.0)

            fjd = None
            for j in range(NT):
                jj = slice(j, j + 1)
                # Act engine: R1 = 1/(ta*rpa+1); By = relu(ty1-py1);
                #             RMy = relu(py1+ph-ty2)
                rS = wp.tile([ROWS, T], f16, name="rS", tag="rS")
                _raw_activation(
                    nc, rS[:], TAh[:], AF.Reciprocal, 1.0, rpa[:, jj]
                )
                By = wp.tile([ROWS, T], f16, name="By", tag="By")
                nc.scalar.activation(
                    out=By[:], in_=TY1h[:], func=AF.Relu, bias=npy1[:, jj], scale=1.0
                )
                RMy = wp.tile([ROWS, T], f16, name="RMy", tag="RMy")
                nc.scalar.activation(
                    out=RMy[:], in_=TY2h[:], func=AF.Relu, bias=pyh[:, jj], scale=-1.0
                )

                # DVE: wr = x-overlap (custom); ysum = RMy+By; wrs = wr*rS
                # f = relu(wrs) * relu(ph - ysum) * rpa  (= inter/(ta+pa))
                wr = wp.tile([ROWS, T], f16, name="wr", tag="wr")
                nc.vector._custom_dve(
                    OV_OP,
                    out=wr[:],
                    in0=TX2h[:],
                    in1=TX1h[:],
                    s0=px1_32[:, jj],
                    s1=params_sb[:, j, 0:1],
                )
                ysum = ppy.tile([ROWS, T], f32, name="ysum", tag="ysum")
                nc.tensor.matmul(
                    out=ysum[:], lhsT=identh[:], rhs=RMy[:], start=True, stop=False
                )
                nc.tensor.matmul(
                    out=ysum[:], lhsT=identh[:], rhs=By[:], start=False, stop=True
                )
                wrs = wp.tile([ROWS, T], f16, name="wrs", tag="wrs")
                nc.vector.tensor_tensor(out=wrs[:], in0=wr[:], in1=rS[:], op=OP.mult)

                if j % 4 == 0:
                    fjd = wp.tile([ROWS, 4, T], f16, name="fjd", tag="fjd")
                fj = fjd[:, j % 4, :]
                nc.vector._custom_dve(
                    FMR3_OP,
                    out=fj,
                    in0=wrs[:],
                    in1=ysum[:],
                    s0=rpa[:, jj],
                    s1=params_sb[:, j, 1:2],
                    imm2=0.0,
                    accum_out=btoh[:, jj],
                )
                nc.vector.max_index(
                    bti8[:, j, :], btoh[:, jj].to_broadcast([ROWS, 8]), fj
                )
                if j % 4 == 3:
                    nc.vector.tensor_tensor(
                        out=runmax4[:],
                        in0=runmax4[:],
                        in1=fjd[:],
                        op=OP.max,
                    )
                nc.gpsimd.indirect_dma_start(
                    out=pay_all[:, j, :],
                    out_offset=None,
                    in_=pay_dram[:],
                    in_offset=bass.IndirectOffsetOnAxis(ap=bti8[:, j, 0:1], axis=0),
                )

            # ---------------- Phase 2a: local best-prior + packed AllReduce ---
            runmax = pe.tile([ROWS, T], f32, name="runmax")
            rmh2 = wp.tile([ROWS, 2, T], f16, name="rmh2", tag="rmh2")
            nc.vector.tensor_tensor(
                out=rmh2[:], in0=runmax4[:, 0:2, :], in1=runmax4[:, 2:4, :], op=OP.max
            )
            rmh = wp.tile([ROWS, T], f16, name="rmh", tag="rmh")
            nc.vector.tensor_tensor(
                out=rmh[:], in0=rmh2[:, 0, :], in1=rmh2[:, 1, :], op=OP.max
            )
            nc.vector.tensor_copy(out=runmax[:], in_=rmh[:])

            B4s = []
            jstars = []
            pgs = []
            vqs = []
            for g in range(2):
                gsl = slice(g * ROWS, (g + 1) * ROWS)
                psT = pp.tile([ROWS, ROWS], f32, name=f"psT{g}", tag="ps")
                nc.tensor.transpose(out=psT[:], in_=runmax[:, gsl], identity=ident[:])
                rmT = pe.tile([ROWS, ROWS], f32, name=f"rmT{g}")
                nc.scalar.copy(out=rmT[:], in_=psT[:])
                # per-t max over q (on this core)
                m8 = pe.tile([ROWS, 8], f32, name=f"m8_{g}")
                nc.vector.max(m8[:], rmT[:])
                i8 = pe.tile([ROWS, 8], u32, name=f"i8_{g}")
                nc.vector.max_index(i8[:], m8[:], rmT[:])
                qstar = pe.tile([ROWS, 1], f32, name=f"qstar{g}")
                nc.vector.tensor_copy(out=qstar[:], in_=i8[:, 0:1])

                # build E[q, t] = (q == q*[t]) via transpose of broadcast
                qb_ps = pp.tile([ROWS, ROWS], f32, name=f"qb_ps{g}", tag="ps")
                nc.tensor.transpose(
                    out=qb_ps[:],
                    in_=qstar[:, 0:1].to_broadcast([ROWS, ROWS]),
                    identity=ident[:],
                )
                qbT = wp.tile([ROWS, ROWS], f32, name=f"qbT{g}", tag="qbT")
                nc.scalar.copy(out=qbT[:], in_=qb_ps[:])
                E = wp.tile([ROWS, ROWS], bf16, name=f"E{g}", tag="E")
                nc.vector.tensor_scalar(
                    out=E[:], in0=qbT[:], scalar1=ridf[:], scalar2=None, op0=OP.is_equal
                )

                # gather prior rows q*[t] via 2 packed one-hot bf16 matmuls
                B4_ps = pp.tile([ROWS, ROWS, 4], f32, name=f"B4_ps{g}", tag="ps")
                nc.tensor.matmul(
                    out=B4_ps[:],
                    lhsT=E[:],
                    rhs=parr[:, :, 0:4],
                    start=True,
                    stop=True,
                )
                B4 = wp.tile([ROWS, ROWS, 4], f32, name=f"B4{g}", tag="B4")
                nc.scalar.copy(out=B4[:], in_=B4_ps[:])
                B1_ps = pp.tile([ROWS, ROWS], f32, name=f"B1_ps{g}", tag="ps")
                nc.tensor.matmul(
                    out=B1_ps[:],
                    lhsT=E[:],
                    rhs=parr[:, :, 4],
                    start=True,
                    stop=True,
                )
                B_ap = wp.tile([ROWS, ROWS], f32, name=f"B_ap{g}", tag="B_ap")
                nc.scalar.copy(out=B_ap[:], in_=B1_ps[:])
                B_px1 = B4[:, :, 0]
                B_pw = B4[:, :, 1]
                B_py1 = B4[:, :, 2]
                B_ph = B4[:, :, 3]

                tc4 = tcols_g[g]
                wr_re = wp.tile([ROWS, ROWS], f32, name=f"wr_re{g}", tag="wr_re")
                nc.vector._custom_dve(
                    OVF_OP,
                    out=wr_re[:],
                    in0=B_px1,
                    in1=B_pw,
                    s0=tc4[:, 2:3],
                    s1=tc4[:, 0:1],
                )
                hr_re = wp.tile([ROWS, ROWS], f32, name=f"hr_re{g}", tag="hr_re")
                nc.vector._custom_dve(
                    OVF_OP,
                    out=hr_re[:],
                    in0=B_py1,
                    in1=B_ph,
                    s0=tc4[:, 3:4],
                    s1=tc4[:, 1:2],
                )
                S_re = wp.tile([ROWS, ROWS], f32, name=f"S_re{g}", tag="S_re")
                nc.vector.tensor_scalar(
                    out=S_re[:], in0=B_ap[:], scalar1=TAc[g][:], scalar2=None, op0=OP.add
                )
                lS_re = wp.tile([ROWS, ROWS], f32, name=f"lS_re{g}", tag="lS_re")
                nc.scalar.activation(out=lS_re[:], in_=S_re[:], func=AF.Ln)
                rS_re = wp.tile([ROWS, ROWS], f32, name=f"rS_re{g}", tag="rS_re")
                nc.scalar.activation(out=rS_re[:], in_=lS_re[:], func=AF.Exp, scale=-1.0)
                f_re = wp.tile([ROWS, ROWS], f32, name=f"f_re{g}", tag="f_re")
                nc.vector.tensor_tensor(
                    out=f_re[:], in0=wr_re[:], in1=rS_re[:], op=OP.mult
                )
                nc.vector.tensor_tensor(
                    out=f_re[:], in0=f_re[:], in1=hr_re[:], op=OP.mult
                )
                mre8 = wp.tile([ROWS, 8], f32, name=f"mre8{g}", tag="mre8")
                nc.vector.max(mre8[:], f_re[:])
                jre8 = wp.tile([ROWS, 8], u32, name=f"jre8{g}", tag="jre8")
                nc.vector.max_index(jre8[:], mre8[:], f_re[:])
                jstar = pe.tile([ROWS, 1], f32, name=f"jstar{g}")
                nc.vector.tensor_copy(out=jstar[:], in_=jre8[:, 0:1])
                B4s.append(B4)
                jstars.append(jstar)

                # local candidate index (within this core) and quantized value;
                # the final pack (with the candidate's xf bit) is assembled
                # after phase 3 stages st3 to DRAM.
                pg = pe.tile([ROWS, 1], f32, name=f"pg{g}")
                nc.vector.tensor_scalar(
                    out=pg[:], in0=qstar[:], scalar1=float(NT), scalar2=None, op0=OP.mult
                )
                nc.vector.tensor_tensor(out=pg[:], in0=pg[:], in1=jstar[:], op=OP.add)
                vqf = wp.tile([ROWS, 1], f32, name=f"vqf{g}", tag="vqf")
                nc.vector.tensor_scalar(
                    out=vqf[:], in0=m8[:, 0:1], scalar1=128.0, scalar2=None, op0=OP.mult
                )
                vqu = wp.tile([ROWS, 1], u32, name=f"vqu{g}", tag="vqu")
                nc.vector.tensor_copy(out=vqu[:], in_=vqf[:])
                vqf2 = pe.tile([ROWS, 1], f32, name=f"vqf2{g}")
                nc.vector.tensor_copy(out=vqf2[:], in_=vqu[:])
                pgs.append(pg)
                vqs.append(vqf2)

            # candidate prior-row data selection + recomputed l1n (as before)
            from concourse.dve_ops import TENSOR_TENSOR_REDUCE as TTR_OP

            gsel = []
            l1ns = []
            for g in range(2):
                ohj = wp.tile([ROWS, ROWS], f32, name=f"ohj{g}", tag="ohj")
                nc.vector.tensor_scalar(
                    out=ohj[:],
                    in0=IOTA_TF[:, 0:ROWS],
                    scalar1=jstars[g][:],
                    scalar2=None,
                    op0=OP.is_equal,
                )
                cols = pe.tile([ROWS, 4], f32, name=f"gselc{g}")
                for c in range(4):
                    trash = wp.tile([ROWS, ROWS], f32, name=f"tsel{g}_{c}", tag="tsel")
                    nc.vector._custom_dve(
                        TTR_OP,
                        out=trash[:],
                        in0=B4s[g][:, :, c],
                        in1=ohj[:],
                        s0=0.0,
                        s1=1.0,
                        accum_out=cols[:, c : c + 1],
                    )
                gsel.append(cols)
                tcols = [PAYg[g][:, c : c + 1] for c in range(4)]
                gw = cols[:, 1:2]
                gh = cols[:, 3:4]
                gcx = pe.tile([ROWS, 1], f32, name=f"gcx{g}")
                nc.vector.tensor_scalar(
                    out=gcx[:], in0=gw, scalar1=0.5, scalar2=None, op0=OP.mult
                )
                nc.vector.tensor_tensor(
                    out=gcx[:], in0=gcx[:], in1=cols[:, 0:1], op=OP.add
                )
                gcy = pe.tile([ROWS, 1], f32, name=f"gcy{g}")
                nc.vector.tensor_scalar(
                    out=gcy[:], in0=gh, scalar1=0.5, scalar2=None, op0=OP.mult
                )
                nc.vector.tensor_tensor(
                    out=gcy[:], in0=gcy[:], in1=cols[:, 2:3], op=OP.add
                )
                rgw = pe.tile([ROWS, 1], f32, name=f"rgw{g}")
                nc.vector.reciprocal(out=rgw[:], in_=gw)
                rgh = pe.tile([ROWS, 1], f32, name=f"rgh{g}")
                nc.vector.reciprocal(out=rgh[:], in_=gh)
                lgw5 = pe.tile([ROWS, 1], f32, name=f"lgw5{g}")
                nc.scalar.activation(out=lgw5[:], in_=gw, func=AF.Ln)
                nc.vector.tensor_scalar(
                    out=lgw5[:], in0=lgw5[:], scalar1=5.0, scalar2=None, op0=OP.mult
                )
                lgh5 = pe.tile([ROWS, 1], f32, name=f"lgh5{g}")
                nc.scalar.activation(out=lgh5[:], in_=gh, func=AF.Ln)
                nc.vector.tensor_scalar(
                    out=lgh5[:], in0=lgh5[:], scalar1=5.0, scalar2=None, op0=OP.mult
                )

                encs = []
                for cc, (tcol, gcen, rg_) in enumerate(
                    ((tcols[0], gcx, rgw), (tcols[1], gcy, rgh))
                ):
                    gx10 = wp.tile([ROWS, 1], f32, name=f"gx10_{g}_{cc}", tag="gx10")
                    nc.vector.tensor_scalar(
                        out=gx10[:], in0=gcen, scalar1=10.0, scalar2=None, op0=OP.mult
                    )
                    en = pe.tile([ROWS, 1], f32, name=f"en{g}_{cc}")
                    nc.vector.tensor_tensor(
                        out=en[:], in0=tcol, in1=gx10[:], op=OP.subtract
                    )
                    nc.vector.tensor_tensor(out=en[:], in0=en[:], in1=rg_[:], op=OP.mult)
                    encs.append(en)
                for cc, (tcol, lg_) in enumerate(((tcols[2], lgw5), (tcols[3], lgh5))):
                    en = pe.tile([ROWS, 1], f32, name=f"en{g}_{cc + 2}")
                    nc.vector.tensor_tensor(
                        out=en[:], in0=tcol, in1=lg_[:], op=OP.subtract
                    )
                    encs.append(en)

                l1n = pe.tile([ROWS, 1], f32, name=f"l1n{g}")
                nc.vector.memset(l1n[:], 0.0)
                for cc, en in enumerate(encs):
                    slv = wp.tile([ROWS, 1], f32, name=f"slv{g}_{cc}", tag="slv")
                    nc.vector._custom_dve(SL1_OP, out=slv[:], in0=en[:], s0=1.0, s1=0.0)
                    nc.vector.tensor_tensor(out=l1n[:], in0=l1n[:], in1=slv[:], op=OP.add)

                l1ns.append(l1n)

            # ---------------- Phase 3: per-prior epilogue ----------------
            xf = pe.tile([ROWS, NT], f32, name="xf")
            nc.vector.tensor_scalar(
                out=xf[:], in0=btoh[:], scalar1=TH_F, scalar2=None, op0=OP.is_gt
            )
            ptcx = pay_all[:, :, 0]
            ptcy = pay_all[:, :, 1]
            ptlw = pay_all[:, :, 2]
            ptlh = pay_all[:, :, 3]
            e_tiles = []
            for idx, (pay, cen, rp) in enumerate(
                ((ptcx, pcx10, rpw), (ptcy, pcy10, rph))
            ):
                e = wp.tile([ROWS, NT], f32, name=f"e{idx}", tag=f"e{idx}")
                nc.vector.tensor_tensor(out=e[:], in0=pay, in1=cen[:], op=OP.subtract)
                nc.vector.tensor_tensor(out=e[:], in0=e[:], in1=rp[:], op=OP.mult)
                e_tiles.append(e)
            for idx, (pay, lp) in enumerate(((ptlw, lpw5), (ptlh, lph5))):
                e = wp.tile([ROWS, NT], f32, name=f"e{idx + 2}", tag=f"e{idx + 2}")
                nc.vector.tensor_tensor(out=e[:], in0=pay, in1=lp[:], op=OP.subtract)
                e_tiles.append(e)

            l1u = pe.tile([ROWS, NT], f32, name="l1u")
            sl_prev = None
            for idx, e in enumerate(e_tiles):
                sl = wp.tile([ROWS, NT], f32, name=f"sl{idx}", tag=f"sl{idx}")
                nc.vector._custom_dve(SL1_OP, out=sl[:], in0=e[:], s0=1.0, s1=0.0)
                if idx == 1:
                    nc.vector.tensor_tensor(
                        out=l1u[:], in0=sl_prev[:], in1=sl[:], op=OP.add
                    )
                elif idx > 1:
                    nc.vector.tensor_tensor(out=l1u[:], in0=l1u[:], in1=sl[:], op=OP.add)
                sl_prev = sl

            term = pe.tile([ROWS, NT], f32, name="term")
            nc.vector.tensor_tensor(out=term[:], in0=s_sb[:], in1=xf[:], op=OP.mult)
            nc.vector.tensor_tensor(out=term[:], in0=term[:], in1=l1u[:], op=OP.mult)
            pack3 = pe.tile([ROWS, 3], f32, name="pack3")
            nc.vector.tensor_reduce(
                out=pack3[:, 0:1], in_=term[:], axis=mybir.AxisListType.X, op=OP.add
            )
            nc.vector.tensor_reduce(
                out=pack3[:, 1:2], in_=xf[:], axis=mybir.AxisListType.X, op=OP.add
            )
            nc.vector.tensor_reduce(
                out=pack3[:, 2:3], in_=s_sb[:], axis=mybir.AxisListType.X, op=OP.add
            )
            sums_ps = pp.tile([1, 3], f32, name="sums_ps", tag="ps")
            nc.tensor.matmul(
                out=sums_ps[:], lhsT=ones128[:], rhs=pack3[:], start=True, stop=True
            )
            sums_sb = pe.tile([1, 3], f32, name="sums_sb")
            nc.scalar.copy(out=sums_sb[:], in_=sums_ps[:])

            # stage per-prior arrays to DRAM for phase-4 indirect gathers
            st3 = pe.tile([ROWS, NT, 3], f32, name="st3")
            nc.scalar.copy(out=st3[:, :, 0], in_=l1u[:])
            nc.scalar.copy(out=st3[:, :, 1], in_=xf[:])
            nc.scalar.copy(out=st3[:, :, 2], in_=s_sb[:])
            st3_dram = dp.tile([PL, 3], f32, name="st3_dram")
            nc.sync.dma_start(
                out=st3_dram[:].rearrange("(r j) c -> r j c", r=ROWS), in_=st3[:]
            )

            # ======== merged endgame: one AllGather carries ========
            #   [0:256)   per-core candidate pack  (qval<<18 | xf<<17 | p_global)
            #   [256:512) per-core candidate dnum  (s*(K*l1n - xf*l1u))
            #   [512:515) per-core partial sums    (S1, S2, S3)
            # every core then unpacks winners, dedups, and computes the loss.
            ag_in = dp.tile([1, 516], f32, name="ag_in")
            for g in range(2):
                phu = pe.tile([ROWS, 1], u32, name=f"phu{g}")
                nc.vector.tensor_copy(out=phu[:], in_=pgs[g][:])
                gat3 = pe.tile([ROWS, 3], f32, name=f"gat3{g}")
                nc.gpsimd.indirect_dma_start(
                    out=gat3[:],
                    out_offset=None,
                    in_=st3_dram[:],
                    in_offset=bass.IndirectOffsetOnAxis(ap=phu[:, 0:1], axis=0),
                )
                l1_at = gat3[:, 0:1]
                xf_at = gat3[:, 1:2]
                s_at = gat3[:, 2:3]

                w1 = wp.tile([ROWS, 1], f32, name=f"w1{g}", tag="w1")
                nc.vector.tensor_scalar(
                    out=w1[:], in0=l1ns[g][:], scalar1=K_VAL, scalar2=None, op0=OP.mult
                )
                w2 = wp.tile([ROWS, 1], f32, name=f"w2{g}", tag="w2")
                nc.vector.tensor_tensor(out=w2[:], in0=xf_at, in1=l1_at, op=OP.mult)
                nc.vector.tensor_tensor(out=w1[:], in0=w1[:], in1=w2[:], op=OP.subtract)
                dn_c = pe.tile([ROWS, 1], f32, name=f"dn_c{g}")
                nc.vector.tensor_tensor(out=dn_c[:], in0=w1[:], in1=s_at, op=OP.mult)

                pk = pe.tile([ROWS, 1], f32, name=f"pk{g}")
                nc.vector.tensor_scalar(
                    out=pk[:],
                    in0=vqs[g][:],
                    scalar1=63.0,
                    scalar2=262144.0,
                    op0=OP.min,
                    op1=OP.mult,
                )
                pxf = wp.tile([ROWS, 1], f32, name=f"pxf{g}", tag="pxf")
                nc.vector.tensor_scalar(
                    out=pxf[:], in0=xf_at, scalar1=131072.0, scalar2=None, op0=OP.mult
                )
                nc.vector.tensor_tensor(out=pk[:], in0=pk[:], in1=pxf[:], op=OP.add)
                nc.vector.tensor_tensor(out=pk[:], in0=pk[:], in1=pgs[g][:], op=OP.add)
                nc.vector.tensor_tensor(out=pk[:], in0=pk[:], in1=POFFb[:], op=OP.add)
                nc.sync.dma_start(
                    out=ag_in[0, g * ROWS : (g + 1) * ROWS], in_=pk[:, 0:1]
                )
                nc.sync.dma_start(
                    out=ag_in[0, 256 + g * ROWS : 256 + (g + 1) * ROWS],
                    in_=dn_c[:, 0:1],
                )
            pad4 = pe.tile([1, 4], f32, name="pad4")
            nc.vector.memset(pad4[:], 0.0)
            nc.vector.tensor_copy(out=pad4[:, 0:3], in_=sums_sb[:])
            nc.sync.dma_start(out=ag_in[0, 512:516], in_=pad4[:])

            ag_out = dp.tile([8, 516], f32, name="ag_out", addr_space="Shared")
            nc.gpsimd.collective_compute(
                "AllGather",
                mybir.AluOpType.bypass,
                ins=[ag_in[:]],
                outs=[ag_out[:]],
                replica_groups=RG,
            )

            # ---- post-collective: winners, dedup, corrections, loss ----
            p_star = []
            xfbs = []
            dnws = []
            for g in range(2):
                gsl = slice(g * ROWS, (g + 1) * ROWS)
                pk8 = pe.tile([ROWS, 8], f32, name=f"pk8_{g}")
                nc.sync.dma_start(
                    out=pk8[:], in_=ag_out[:, gsl].rearrange("l t -> t l")
                )
                win = pe.tile([ROWS, 1], f32, name=f"win{g}")
                nc.vector.tensor_reduce(
                    out=win[:], in_=pk8[:], axis=mybir.AxisListType.X, op=OP.max
                )
                # unpack: qval = floor(win/2^18); rem = win - qval*2^18
                vq1 = wp.tile([ROWS, 1], f32, name=f"vq1{g}", tag="vq1")
                nc.vector.tensor_scalar(
                    out=vq1[:], in0=win[:], scalar1=1.0 / 262144.0, scalar2=None,
                    op0=OP.mult,
                )
                vq1u = wp.tile([ROWS, 1], u32, name=f"vq1u{g}", tag="vq1u")
                nc.vector.tensor_copy(out=vq1u[:], in_=vq1[:])
                vq1f = wp.tile([ROWS, 1], f32, name=f"vq1f{g}", tag="vq1f")
                nc.vector.tensor_copy(out=vq1f[:], in_=vq1u[:])
                # the f32->u32 copy rounds to nearest, so qv may be off by
                # one; recover rem = win mod 2^18 with a sign fixup, then the
                # same for the xf bit / prior-id split.
                rem = pe.tile([ROWS, 1], f32, name=f"rem{g}")
                nc.vector.tensor_scalar(
                    out=rem[:], in0=vq1f[:], scalar1=-262144.0, scalar2=None,
                    op0=OP.mult,
                )
                nc.vector.tensor_tensor(out=rem[:], in0=rem[:], in1=win[:], op=OP.add)
                fx1 = wp.tile([ROWS, 1], f32, name=f"fx1{g}", tag="fx1")
                nc.vector.tensor_scalar(
                    out=fx1[:], in0=rem[:], scalar1=0.0, scalar2=262144.0,
                    op0=OP.is_lt, op1=OP.mult,
                )
                nc.vector.tensor_tensor(out=rem[:], in0=rem[:], in1=fx1[:], op=OP.add)

                xb1 = wp.tile([ROWS, 1], f32, name=f"xb1{g}", tag="xb1")
                nc.vector.tensor_scalar(
                    out=xb1[:], in0=rem[:], scalar1=1.0 / 131072.0, scalar2=None,
                    op0=OP.mult,
                )
                xb1u = wp.tile([ROWS, 1], u32, name=f"xb1u{g}", tag="xb1u")
                nc.vector.tensor_copy(out=xb1u[:], in_=xb1[:])
                xfbr = wp.tile([ROWS, 1], f32, name=f"xfbr{g}", tag="xfbr")
                nc.vector.tensor_copy(out=xfbr[:], in_=xb1u[:])
                ps_col = pe.tile([ROWS, 1], f32, name=f"ps_col{g}")
                nc.vector.tensor_scalar(
                    out=ps_col[:], in0=xfbr[:], scalar1=-131072.0, scalar2=None,
                    op0=OP.mult,
                )
                nc.vector.tensor_tensor(
                    out=ps_col[:], in0=ps_col[:], in1=rem[:], op=OP.add
                )
                neg2 = wp.tile([ROWS, 1], f32, name=f"neg2{g}", tag="neg2")
                nc.vector.tensor_scalar(
                    out=neg2[:], in0=ps_col[:], scalar1=0.0, scalar2=None,
                    op0=OP.is_lt,
                )
                fx2 = wp.tile([ROWS, 1], f32, name=f"fx2{g}", tag="fx2")
                nc.vector.tensor_scalar(
                    out=fx2[:], in0=neg2[:], scalar1=131072.0, scalar2=None,
                    op0=OP.mult,
                )
                nc.vector.tensor_tensor(
                    out=ps_col[:], in0=ps_col[:], in1=fx2[:], op=OP.add
                )
                xfb = pe.tile([ROWS, 1], f32, name=f"xfb{g}")
                nc.vector.tensor_tensor(
                    out=xfb[:], in0=xfbr[:], in1=neg2[:], op=OP.subtract
                )

                d8 = pe.tile([ROWS, 8], f32, name=f"d8_{g}")
                nc.sync.dma_start(
                    out=d8[:],
                    in_=ag_out[:, 256 + g * ROWS : 256 + (g + 1) * ROWS].rearrange(
                        "l t -> t l"
                    ),
                )
                # exact lane one-hot from thresholds: oh[l] = (l*PL <= p) - ((l+1)*PL <= p)
                le9 = wp.tile([ROWS, 9], f32, name=f"le9{g}", tag="le9")
                nc.vector.tensor_scalar(
                    out=le9[:], in0=lthrf[:], scalar1=ps_col[:], scalar2=None,
                    op0=OP.is_le,
                )
                oh8 = wp.tile([ROWS, 8], f32, name=f"oh8{g}", tag="oh8")
                nc.vector.tensor_tensor(
                    out=oh8[:], in0=le9[:, 0:8], in1=le9[:, 1:9], op=OP.subtract
                )
                dsel = wp.tile([ROWS, 8], f32, name=f"dsel{g}", tag="dsel")
                nc.vector.tensor_tensor(out=dsel[:], in0=oh8[:], in1=d8[:], op=OP.mult)
                dnw = pe.tile([ROWS, 1], f32, name=f"dnw{g}")
                nc.vector.tensor_reduce(
                    out=dnw[:], in_=dsel[:], axis=mybir.AxisListType.X, op=OP.add
                )
                p_star.append(ps_col)
                xfbs.append(xfb)
                dnws.append(dnw)

            # dedup: a prior claimed by several truths keeps only the last t
            PSb = pe.tile([ROWS, T], f32, name="PSb")
            for g in range(2):
                psb_ps = pp.tile([ROWS, ROWS], f32, name=f"psb_ps{g}", tag="ps")
                nc.tensor.transpose(
                    out=psb_ps[:],
                    in_=p_star[g][:, 0:1].to_broadcast([ROWS, ROWS]),
                    identity=ident[:],
                )
                nc.scalar.copy(out=PSb[:, g * ROWS : (g + 1) * ROWS], in_=psb_ps[:])

            keep = []
            for g in range(2):
                eqm = wp.tile([ROWS, T], f32, name=f"eqm{g}", tag="eqm")
                nc.vector.tensor_scalar(
                    out=eqm[:],
                    in0=PSb[:],
                    scalar1=p_star[g][:, 0:1],
                    scalar2=None,
                    op0=OP.is_equal,
                )
                rid_g = pe.tile([ROWS, 1], f32, name=f"rid_g{g}")
                nc.vector.tensor_scalar(
                    out=rid_g[:],
                    in0=ridf[:],
                    scalar1=float(g * ROWS),
                    scalar2=None,
                    op0=OP.add,
                )
                trg = wp.tile([ROWS, T], f32, name=f"trg{g}", tag="trg")
                nc.vector.tensor_scalar(
                    out=trg[:],
                    in0=IOTA_TF[:],
                    scalar1=rid_g[:],
                    scalar2=None,
                    op0=OP.is_gt,
                )
                anyl = pe.tile([ROWS, 1], f32, name=f"anyl{g}")
                trash3 = wp.tile([ROWS, T], f32, name=f"trash3{g}", tag="trash3")
                nc.vector.tensor_tensor(out=trash3[:], in0=eqm[:], in1=trg[:], op=OP.mult)
                nc.vector.tensor_reduce(
                    out=anyl[:], in_=trash3[:], axis=mybir.AxisListType.X, op=OP.max
                )
                kp = pe.tile([ROWS, 1], f32, name=f"keep{g}")
                nc.vector.tensor_scalar(
                    out=kp[:],
                    in0=anyl[:],
                    scalar1=-1.0,
                    scalar2=1.0,
                    op0=OP.mult,
                    op1=OP.add,
                )
                keep.append(kp)

            dn_g = []
            dd_g = []
            for g in range(2):
                dn = pe.tile([ROWS, 1], f32, name=f"dn{g}")
                nc.vector.tensor_tensor(
                    out=dn[:], in0=dnws[g][:], in1=keep[g][:], op=OP.mult
                )
                dd = pe.tile([ROWS, 1], f32, name=f"dd{g}")
                nc.vector.tensor_scalar(
                    out=dd[:],
                    in0=xfbs[g][:],
                    scalar1=-1.0,
                    scalar2=K_VAL,
                    op0=OP.mult,
                    op1=OP.add,
                )
                nc.vector.tensor_tensor(out=dd[:], in0=dd[:], in1=keep[g][:], op=OP.mult)
                dn_g.append(dn)
                dd_g.append(dd)

            pack2 = pe.tile([ROWS, 2], f32, name="pack2")
            nc.vector.tensor_tensor(
                out=pack2[:, 0:1], in0=dn_g[0][:], in1=dn_g[1][:], op=OP.add
            )
            nc.vector.tensor_tensor(
                out=pack2[:, 1:2], in0=dd_g[0][:], in1=dd_g[1][:], op=OP.add
            )
            sums2_ps = pp.tile([1, 2], f32, name="sums2_ps", tag="ps")
            nc.tensor.matmul(
                out=sums2_ps[:], lhsT=ones128[:], rhs=pack2[:], start=True, stop=True
            )
            sums2_sb = pe.tile([1, 2], f32, name="sums2_sb")
            nc.scalar.copy(out=sums2_sb[:], in_=sums2_ps[:])

            # global S1..S3: sum the 8 gathered lanes
            s83 = pe.tile([8, 3], f32, name="s83")
            nc.sync.dma_start(out=s83[:], in_=ag_out[:, 512:515])
            sg_ps = pp.tile([1, 3], f32, name="sg_ps", tag="ps")
            nc.tensor.matmul(
                out=sg_ps[:], lhsT=ones128[0:8, 0:1], rhs=s83[:], start=True, stop=True
            )
            sumsg = pe.tile([1, 3], f32, name="sumsg")
            nc.scalar.copy(out=sumsg[:], in_=sg_ps[:])

            num = pe.tile([1, 1], f32, name="num")
            nc.vector.tensor_tensor(
                out=num[:], in0=sumsg[:, 0:1], in1=sums2_sb[:, 0:1], op=OP.add
            )
            nc.vector.tensor_scalar(
                out=num[:], in0=num[:], scalar1=0.5, scalar2=None, op0=OP.mult
            )
            nc.vector.tensor_tensor(
                out=num[:], in0=num[:], in1=sumsg[:, 2:3], op=OP.add
            )
            den = pe.tile([1, 1], f32, name="den")
            nc.vector.tensor_tensor(
                out=den[:], in0=sumsg[:, 1:2], in1=sums2_sb[:, 1:2], op=OP.add
            )
            rden = pe.tile([1, 1], f32, name="rden")
            nc.vector.reciprocal(out=rden[:], in_=den[:])
            loss = pe.tile([1, 1], f32, name="loss")
            nc.vector.tensor_tensor(out=loss[:], in0=num[:], in1=rden[:], op=OP.mult)
            nc.sync.dma_start(out=out_ext[:], in_=loss[:])

    from concourse import mybir as _mb

    _mb.codegen_inst_isa_subclasses(nc)
    _split_waits(nc)
    return nc


def _split_waits(nc):
    """This toolchain's codegen accepts only one embedded sem-wait per
    instruction; hoist extra waits into standalone EventSemaphore
    instructions on the same engine (same blocking semantics)."""
    import orjson

    import copy as _copy

    d = orjson.loads(nc.to_json_bytes())
    ctr = 0
    for fn in d.get("functions", []):
        for bb in fn.get("blocks", []):
            out = []
            for ins in bb.get("instructions", []):
                if (
                    ins.get("opcode") == "ISA"
                    and ins.get("op_name") == "EVENT_SEMAPHORE_RANGE_CLEAR"
                ):
                    # codegen rejects clear ranges wider than 16 sems; split.
                    first, last = ins["instr"][13], ins["instr"][14]
                    if last - first + 1 > 16:
                        lo = first
                        while lo <= last:
                            hi = min(lo + 15, last)
                            ctr += 1
                            part = _copy.deepcopy(ins)
                            part["name"] = f"{ins['name']}_rc{ctr}"
                            part["instr"] = list(ins["instr"])
                            part["instr"][13] = lo
                            part["instr"][14] = hi
                            if lo != first:
                                part["sync_info"] = {"on_wait": [], "on_update": []}
                            out.append(part)
                            lo = hi + 1
                        continue
                si = ins.get("sync_info")
                ow = (si or {}).get("on_wait") or []
                if si and len(ow) > 1 and "engine" in ins:
                    for w in ow[:-1]:
                        ctr += 1
                        ev = {
                            "engine": ins["engine"],
                            "ins": [],
                            "outs": [],
                            "name": f"antsplit_{ctr}",
                            "opcode": "EventSemaphore",
                            "sync_info": {"on_wait": [w], "on_update": []},
                        }
                        if "debug" in ins:
                            ev["debug"] = ins["debug"]
                        out.append(ev)
                    si["on_wait"] = [ow[-1]]
                out.append(ins)
            bb["instructions"] = out
    blob = orjson.dumps(d)
    nc.to_json_bytes = lambda: blob
    return nc


def kernel(**inputs):
    locs = np.ascontiguousarray(np.asarray(inputs["locs"], dtype=np.float32))
    params = np.ascontiguousarray(np.asarray(inputs["params"], dtype=np.float32))
    truths = np.ascontiguousarray(np.asarray(inputs["truths"], dtype=np.float32))
    truths4 = np.ascontiguousarray(truths.T)

    if "nc" not in _CACHE:
        _CACHE["nc"] = _build()
    nc = _CACHE["nc"]

    in_maps = []
    for c in range(NCORES):
        in_maps.append(
            {
                "locs": locs[c * PL : (c + 1) * PL],
                "params": params[c * PL : (c + 1) * PL],
                "truths4": truths4,
                "poff": np.array([[c * PL]], dtype=np.float32),
            }
        )

    from concourse.bass_utils import run_bass_kernel_spmd

    res = run_bass_kernel_spmd(nc, in_maps, core_ids=list(range(NCORES)))
    out = np.asarray(res.results[0]["out"], dtype=np.float32)
    return out.reshape(())


if __name__ == "__main__":
    sys.path.insert(0, "/root/problem")
    import reference

    inputs = {k: np.asarray(v) for k, v in reference.setup_inputs().items()}
    expected = np.asarray(reference.reference(**inputs))
    actual = kernel(**inputs)
    rel = abs(float(actual) - float(expected)) / max(abs(float(expected)), 1e-12)
    print("expected:", expected, "actual:", actual, "rel_err:", rel)



# revision 4
# speedup vs baseline: 1.0746x; 1.0024x over previous
import sys

sys.path.insert(0, "/opt/trn_rl_repo")

import numpy as np

# Problem constants (hardcoded per spec nn_AdaptivePriorBoxesLoss)
P_TOT = 131072
T = 256
NCORES = 8
PL = P_TOT // NCORES  # 16384 priors per core
ROWS = 128
NT = PL // ROWS  # 128 tiles per core; local prior p = q*NT + j
TH_F = 2.0 / 7.0  # iou > 0.4  <=>  inter/(areaA+areaB) > 2/7
K_VAL = 2.5
BETA = 1.0

_CACHE = {}


# ---------------------------------------------------------------------------
# Custom DVE ops (registered into concourse.dve_ops process-locally)
# ---------------------------------------------------------------------------
def _register_custom_ops():
    from concourse import dve_ops
    from concourse.dve_ops import DveOp
    from concourse.dve_spec import (
        Spec,
        Src0,
        Src1,
        C0,
        C1,
        C2,
        Zero,
        relu,
        minn,
        maxx,
        lower,
        _has_src1,
    )
    from concourse.dve_uop import DveOpSpec
    from operator import add as _add

    def mk(name, spec):
        if name in dve_ops._SUB_OPCODE_FOR_NAME:
            for op in dve_ops.OPS:
                if op.name == name:
                    return op
        row = dve_ops._CUSTOM_DVE_ROW_BASE + len(dve_ops.OPS)
        shas = {}
        for ver in ("v3", "v4"):
            u = lower(spec, ver=ver)
            ds = DveOpSpec(name=name, opcode=row, uops=u, rd1_en=_has_src1(spec))
            shas[ver] = ds.sha(ver)
        op = DveOp(name, spec, subdim=False, uops_sha=shas)
        dve_ops.OPS.append(op)
        dve_ops._SUB_OPCODE_FOR_NAME[name] = row
        dve_ops.CUSTOM_DVE_SPECS[name] = spec
        return op

    # interval overlap: out = relu(min(Src0 - s0, s1) - relu(Src1 - s0))
    #  = relu(min(tx2 - px1, pw) - relu(tx1 - px1))  [per-truth free dim]
    def _ov_ref(in0, in1, s0, s1, imm2):
        a = np.minimum(in0.astype(np.float32) - s0, s1)
        b = np.maximum(in1.astype(np.float32) - s0, 0.0)
        return np.maximum(a - b, 0.0).astype(np.float32)

    ov = mk(
        "OVERLAP_LEN_ANT",
        Spec(body=relu(minn(Src0 - C0, C1) - relu(Src1 - C0)), reference=_ov_ref),
    )

    # flipped roles (tensor = prior arrays, scalars = truth coords):
    # out = relu(min(s0 - Src0, Src1) - relu(s1 - Src0))
    def _ovf_ref(in0, in1, s0, s1, imm2):
        a = np.minimum(s0 - in0.astype(np.float32), in1)
        b = np.maximum(s1 - in0.astype(np.float32), 0.0)
        return np.maximum(a - b, 0.0).astype(np.float32)

    ovf = mk(
        "OVERLAP_FLIP_ANT",
        Spec(body=relu(minn(C0 - Src0, Src1) - relu(C1 - Src0)), reference=_ovf_ref),
    )

    # fused elementwise product + running-max accumulation
    def _fm_ref(in0, in1, s0, s1, imm2):
        b = (in0.astype(np.float32) * in1).astype(np.float32)
        return b, np.maximum(
            b.reshape(b.shape[0], -1).max(axis=-1, keepdims=True),
            np.asarray(s0, np.float32).reshape(-1, 1),
        )

    fm = mk(
        "FMUL_MAX_ANT",
        Spec(body=Src0 * Src1, accum=maxx, accum_init=C0, reference=_fm_ref),
    )

    # smooth-l1 core: a=|x|; out = min(a,s0)*a + relu(a-s0)   (s0=1.0)
    def _sl1_ref(in0, in1, s0, s1, imm2):
        aa = np.abs(in0.astype(np.float32))
        return (np.minimum(aa, s0) * aa + np.maximum(aa - s0, 0.0)).astype(np.float32)

    a_expr = maxx(Src0, Zero - Src0)
    sl1 = mk(
        "SMOOTHL1_ANT",
        Spec(body=minn(a_expr, C0) * a_expr + relu(a_expr - C0), reference=_sl1_ref),
    )

    # fused final product: out = relu(in0) * relu(in1) * s0, with running-max
    # accumulation (seeded from s1) into accum_out
    def _fmr_ref(in0, in1, s0, s1, imm2):
        b = (
            np.maximum(in0.astype(np.float32), 0.0)
            * np.maximum(in1.astype(np.float32), 0.0)
            * np.asarray(s0, np.float32).reshape(-1, 1)
        ).astype(np.float32)
        return b, np.maximum(
            b.reshape(b.shape[0], -1).max(axis=-1, keepdims=True),
            np.asarray(s1, np.float32).reshape(-1, 1),
        )

    fmr = mk(
        "FMULRELU_MAX_ANT",
        Spec(
            body=relu(Src0) * relu(Src1) * C0,
            accum=maxx,
            accum_init=C1,
            reference=_fmr_ref,
        ),
    )

    # f = relu(in0) * relu(s1 - in1) * s0, running-max accum (init imm2)
    def _fmr3_ref(in0, in1, s0, s1, imm2):
        b = (
            np.maximum(in0.astype(np.float32), 0.0)
            * np.maximum(
                np.asarray(s1, np.float32).reshape(-1, 1) - in1.astype(np.float32),
                0.0,
            )
            * np.asarray(s0, np.float32).reshape(-1, 1)
        ).astype(np.float32)
        return b, np.maximum(
            b.reshape(b.shape[0], -1).max(axis=-1, keepdims=True),
            np.asarray(imm2, np.float32),
        )

    fmr3 = mk(
        "FMR3_ANT",
        Spec(
            body=relu(Src0) * relu(C1 - Src1) * C0,
            accum=maxx,
            accum_init=C2,
            reference=_fmr3_ref,
        ),
    )
    return ov, ovf, fm, sl1, fmr, fmr3


def _raw_activation(nc, out, in_, func, bias_imm, scale_ap):
    """nc.scalar.activation minus the Reciprocal guard: out = func(in*scale+bias).
    bias is a float immediate (required for Reciprocal), scale a [P,1] AP."""
    from concourse import mybir

    inputs = [
        nc.scalar.lower_ap(in_),
        mybir.ImmediateValue(dtype=mybir.dt.float32, value=float(bias_imm)),
        nc.scalar.lower_ap(scale_ap),
        mybir.ImmediateValue(dtype=mybir.dt.float32, value=0.0),
    ]
    return nc.scalar.add_instruction(
        mybir.InstActivation(
            name=nc.get_next_instruction_name(),
            func=func,
            ins=inputs,
            outs=[nc.scalar.lower_ap(out)],
        )
    )


def _build():
    from concourse import bass, mybir, tile
    from concourse.masks import make_identity

    OV_OP, OVF_OP, FM_OP, SL1_OP, FMR_OP, FMR3_OP = _register_custom_ops()

    f32 = mybir.dt.float32
    f16 = mybir.dt.float16
    i32 = mybir.dt.int32
    u32 = mybir.dt.uint32
    AF = mybir.ActivationFunctionType
    OP = mybir.AluOpType

    nc = bass.Bass()
    locs_ext = nc.declare_dram_parameter("locs", [PL, 2], f32, isOutput=False)
    params_ext = nc.declare_dram_parameter("params", [PL, 3], f32, isOutput=False)
    truths4_ext = nc.declare_dram_parameter("truths4", [4, T], f32, isOutput=False)
    poff_ext = nc.declare_dram_parameter("poff", [1, 1], f32, isOutput=False)
    out_ext = nc.declare_dram_parameter("out", [1, 1], f32, isOutput=True)

    RG = [list(range(NCORES))]

    with tile.TileContext(nc) as tc:
        with (
            tc.tile_pool(name="persist", bufs=1) as pe,
            tc.tile_pool(name="work", bufs=3) as wp,
            tc.tile_pool(name="loopw", bufs=10) as lw,
            tc.tile_pool(name="psum", bufs=3, space="PSUM") as pp,
            tc.tile_pool(name="psumy", bufs=3, space="PSUM") as ppy,
            tc.tile_pool(name="dram", bufs=1, space="DRAM") as dp,
        ):
            # ---------------- Phase 0: prep ----------------
            locs_sb = pe.tile([ROWS, NT, 2], f32, name="locs_sb")
            params_sb = pe.tile([ROWS, NT, 3], f32, name="params_sb")
            nc.sync.dma_start(
                out=locs_sb[:], in_=locs_ext[:].rearrange("(r j) c -> r j c", r=ROWS)
            )
            nc.sync.dma_start(
                out=params_sb[:],
                in_=params_ext[:].rearrange("(r j) c -> r j c", r=ROWS),
            )

            ones128 = pe.tile([ROWS, 1], f32, name="ones128")
            nc.vector.memset(ones128[:], 1.0)
            ident = pe.tile([ROWS, ROWS], f32, name="ident")
            make_identity(nc, ident[:])
            identh = pe.tile([ROWS, ROWS], f16, name="identh")
            nc.vector.tensor_copy(out=identh[:], in_=ident[:])

            # sigmoid first: keeps the ACT table switches to 2 total
            s_sb = pe.tile([ROWS, NT], f32, name="s_sb")
            nc.scalar.activation(
                out=s_sb[:], in_=params_sb[:, :, 2], func=AF.Sigmoid
            )

            # truth broadcast tiles (f32) via row-broadcast DMA
            TX1b = pe.tile([ROWS, T], f32, name="TX1b")
            TY1b = pe.tile([ROWS, T], f32, name="TY1b")
            TX2b = pe.tile([ROWS, T], f32, name="TX2b")
            TY2b = pe.tile([ROWS, T], f32, name="TY2b")
            for rix, bt in enumerate((TX1b, TY1b, TX2b, TY2b)):
                nc.sync.dma_start(
                    out=bt[:], in_=truths4_ext[rix : rix + 1, :].to_broadcast([ROWS, T])
                )
            POFFb = pe.tile([ROWS, 1], f32, name="POFFb")
            nc.sync.dma_start(out=POFFb[:], in_=poff_ext[:].to_broadcast([ROWS, 1]))

            # truth columns per 128-chunk: t on partitions (for endgame)
            # tcol[g][:, 0:4] = (tx1, ty1, tx2, ty2)[t],  from strided DMA
            tcols_g = []
            for g in range(2):
                tc4 = pe.tile([ROWS, 4], f32, name=f"tcols{g}")
                nc.sync.dma_start(
                    out=tc4[:],
                    in_=truths4_ext[:, g * ROWS : (g + 1) * ROWS].rearrange(
                        "c t -> t c"
                    ),
                )
                tcols_g.append(tc4)

            # fp16 truth tiles + derived
            TX1h = pe.tile([ROWS, T], f16, name="TX1h")
            TX2h = pe.tile([ROWS, T], f16, name="TX2h")
            TY1h = pe.tile([ROWS, T], f16, name="TY1h")
            TY2h = pe.tile([ROWS, T], f16, name="TY2h")
            nc.vector.tensor_copy(out=TX1h[:], in_=TX1b[:])
            nc.vector.tensor_copy(out=TX2h[:], in_=TX2b[:])
            nc.vector.tensor_copy(out=TY1h[:], in_=TY1b[:])
            nc.vector.tensor_copy(out=TY2h[:], in_=TY2b[:])
            TW32 = pe.tile([ROWS, T], f32, name="TW32")
            TH32 = pe.tile([ROWS, T], f32, name="TH32")
            TA32 = pe.tile([ROWS, T], f32, name="TA32")
            nc.vector.tensor_tensor(out=TW32[:], in0=TX2b[:], in1=TX1b[:], op=OP.subtract)
            nc.vector.tensor_tensor(out=TH32[:], in0=TY2b[:], in1=TY1b[:], op=OP.subtract)
            nc.vector.tensor_tensor(out=TA32[:], in0=TW32[:], in1=TH32[:], op=OP.mult)
            TAh = pe.tile([ROWS, T], f16, name="TAh")
            nc.vector.tensor_copy(out=TAh[:], in_=TA32[:])

            # payload table PAY[t, c] = (10*tcx, 10*tcy, 5*ln tw, 5*ln th)
            # as per-128-chunk columns [t-part, 4] and a DRAM copy for gathers
            pay_dram = dp.tile([T, 4], f32, name="pay_dram")
            PAYg = []
            for g in range(2):
                tc4 = tcols_g[g]
                pay_t = pe.tile([ROWS, 4], f32, name=f"PAY{g}")
                # 10*tcx = 5*(tx1+tx2), 10*tcy = 5*(ty1+ty2)
                nc.vector.tensor_tensor(
                    out=pay_t[:, 0:1], in0=tc4[:, 0:1], in1=tc4[:, 2:3], op=OP.add
                )
                nc.vector.tensor_tensor(
                    out=pay_t[:, 1:2], in0=tc4[:, 1:2], in1=tc4[:, 3:4], op=OP.add
                )
                nc.vector.tensor_scalar(
                    out=pay_t[:, 0:2], in0=pay_t[:, 0:2], scalar1=5.0, scalar2=None,
                    op0=OP.mult,
                )
                twc = wp.tile([ROWS, 2], f32, name=f"twc{g}", tag="twc")
                nc.vector.tensor_tensor(
                    out=twc[:, 0:1], in0=tc4[:, 2:3], in1=tc4[:, 0:1], op=OP.subtract
                )
                nc.vector.tensor_tensor(
                    out=twc[:, 1:2], in0=tc4[:, 3:4], in1=tc4[:, 1:2], op=OP.subtract
                )
                nc.scalar.activation(out=pay_t[:, 2:4], in_=twc[:], func=AF.Ln)
                nc.vector.tensor_scalar(
                    out=pay_t[:, 2:4], in0=pay_t[:, 2:4], scalar1=5.0, scalar2=None,
                    op0=OP.mult,
                )
                nc.sync.dma_start(out=pay_dram[g * ROWS : (g + 1) * ROWS, :], in_=pay_t[:])
                PAYg.append(pay_t)
            # truth area column per chunk: ta = tw*th
            TAc = []
            for g in range(2):
                tc4 = tcols_g[g]
                tac = pe.tile([ROWS, 1], f32, name=f"TAc{g}")
                tw1 = wp.tile([ROWS, 1], f32, name=f"tw1{g}", tag="tw1")
                th1 = wp.tile([ROWS, 1], f32, name=f"th1{g}", tag="th1")
                nc.vector.tensor_tensor(
                    out=tw1[:], in0=tc4[:, 2:3], in1=tc4[:, 0:1], op=OP.subtract
                )
                nc.vector.tensor_tensor(
                    out=th1[:], in0=tc4[:, 3:4], in1=tc4[:, 1:2], op=OP.subtract
                )
                nc.vector.tensor_tensor(out=tac[:], in0=tw1[:], in1=th1[:], op=OP.mult)
                TAc.append(tac)

            lthr = pe.tile([ROWS, 9], i32, name="lthr")
            nc.gpsimd.iota(lthr[:], [[1, 9]], base=0, channel_multiplier=0)
            lthrf = pe.tile([ROWS, 9], f32, name="lthrf")
            nc.vector.tensor_copy(out=lthrf[:], in_=lthr[:])
            nc.vector.tensor_scalar(
                out=lthrf[:], in0=lthrf[:], scalar1=float(PL), scalar2=None,
                op0=OP.mult,
            )

            # iotas
            it_i = pe.tile([ROWS, T], i32, name="it_i")
            nc.gpsimd.iota(it_i[:], [[1, T]], base=0, channel_multiplier=0)
            IOTA_TF = pe.tile([ROWS, T], f32, name="IOTA_TF")
            nc.vector.tensor_copy(out=IOTA_TF[:], in_=it_i[:])
            rid_i = pe.tile([ROWS, 1], i32, name="rid_i")
            nc.gpsimd.iota(rid_i[:], [[0, 1]], base=0, channel_multiplier=1)
            ridf = pe.tile([ROWS, 1], f32, name="ridf")
            nc.vector.tensor_copy(out=ridf[:], in_=rid_i[:])

            # per-prior derived arrays
            cxv = locs_sb[:, :, 0]
            cyv = locs_sb[:, :, 1]
            wv = params_sb[:, :, 0]
            hv = params_sb[:, :, 1]
            av = params_sb[:, :, 2]

            px1_32 = pe.tile([ROWS, NT], f32, name="px1_32")
            py1_32 = pe.tile([ROWS, NT], f32, name="py1_32")
            pw_32 = pe.tile([ROWS, NT], f32, name="pw_32")
            ph_32 = pe.tile([ROWS, NT], f32, name="ph_32")
            areap = pe.tile([ROWS, NT], f32, name="areap")
            pcx10 = pe.tile([ROWS, NT], f32, name="pcx10")
            pcy10 = pe.tile([ROWS, NT], f32, name="pcy10")
            rpw = pe.tile([ROWS, NT], f32, name="rpw")
            rph = pe.tile([ROWS, NT], f32, name="rph")
            lpw5 = pe.tile([ROWS, NT], f32, name="lpw5")
            lph5 = pe.tile([ROWS, NT], f32, name="lph5")

            halfw = wp.tile([ROWS, NT], f32, name="halfw", tag="halfw")
            halfh = wp.tile([ROWS, NT], f32, name="halfh", tag="halfh")
            nc.vector.tensor_scalar(
                out=halfw[:], in0=wv, scalar1=0.5, scalar2=None, op0=OP.mult
            )
            nc.vector.tensor_scalar(
                out=halfh[:], in0=hv, scalar1=0.5, scalar2=None, op0=OP.mult
            )
            nc.vector.tensor_tensor(out=px1_32[:], in0=cxv, in1=halfw[:], op=OP.subtract)
            nc.vector.tensor_tensor(out=py1_32[:], in0=cyv, in1=halfh[:], op=OP.subtract)
            nc.scalar.copy(out=pw_32[:], in_=wv)
            nc.scalar.copy(out=ph_32[:], in_=hv)
            nc.vector.tensor_tensor(out=areap[:], in0=wv, in1=hv, op=OP.mult)
            nc.vector.tensor_scalar(
                out=pcx10[:], in0=cxv, scalar1=10.0, scalar2=None, op0=OP.mult
            )
            nc.vector.tensor_scalar(
                out=pcy10[:], in0=cyv, scalar1=10.0, scalar2=None, op0=OP.mult
            )
            nc.vector.reciprocal(out=rpw[:], in_=wv)
            nc.vector.reciprocal(out=rph[:], in_=hv)
            rpa = pe.tile([ROWS, NT], f32, name="rpa")
            nc.vector.reciprocal(out=rpa[:], in_=areap[:])
            npy1 = pe.tile([ROWS, NT], f32, name="npy1")
            nc.vector.tensor_scalar(
                out=npy1[:], in0=py1_32[:], scalar1=-1.0, scalar2=None, op0=OP.mult
            )
            # pyh = py1 + ph (for RMy = relu(pyh - ty2))
            pyh = pe.tile([ROWS, NT], f32, name="pyh")
            nc.vector.tensor_tensor(out=pyh[:], in0=py1_32[:], in1=hv, op=OP.add)
            nc.scalar.activation(out=lpw5[:], in_=wv, func=AF.Ln)
            nc.vector.tensor_scalar(
                out=lpw5[:], in0=lpw5[:], scalar1=5.0, scalar2=None, op0=OP.mult
            )
            nc.scalar.activation(out=lph5[:], in_=hv, func=AF.Ln)
            nc.vector.tensor_scalar(
                out=lph5[:], in0=lph5[:], scalar1=5.0, scalar2=None, op0=OP.mult
            )

            # packed prior arrays (bf16) for the endgame one-hot gathers
            bf16 = mybir.dt.bfloat16
            parr = pe.tile([ROWS, NT, 5], bf16, name="parr")
            nc.vector.tensor_copy(out=parr[:, :, 0], in_=px1_32[:])
            nc.vector.tensor_copy(out=parr[:, :, 1], in_=pw_32[:])
            nc.vector.tensor_copy(out=parr[:, :, 2], in_=py1_32[:])
            nc.vector.tensor_copy(out=parr[:, :, 3], in_=ph_32[:])
            nc.vector.tensor_copy(out=parr[:, :, 4], in_=areap[:])

            # ---------------- Phase 1: main IoU loop ----------------
            # Per j (128 priors x 256 truths):
            #   Act : R1 = 1/(ta*rpa_j + 1)            [reciprocal table]
            #   DVE : Mx = (tx2 - px1) min pw          [ts @4x]
            #         Bx = (tx1 - px1) max 0           [ts @4x]
            #         My = (ty2 - py1) min ph          [ts @4x]
            #         By = (ty1 - py1) max 0           [ts @4x]
            #   Pool: wxu = Mx - Bx, wyu = My - By, wrs = wyu * R1
            #   DVE : f = relu(wxu)*relu(wrs)*rpa_j  (+ max-accum -> btoh)
            #         max_index -> bti8
            #   Pool: runmax over j pairs
            # f == inter/(ta+pa) exactly as before since
            #   rpa/(ta*rpa+1) == 1/(ta+pa).
            btoh = pe.tile([ROWS, NT], f16, name="btoh")
            bti8 = pe.tile([ROWS, NT, 8], u32, name="bti8")
            pay_all = pe.tile([ROWS, NT, 4], f32, name="pay_all")
            runmax4 = pe.tile([ROWS, 4, T], f16, name="runmax4")
            nc.vector.memset(runmax4[:], 0.0)

            fjd = None
            for j in range(NT):
                jj = slice(j, j + 1)
                # Act engine: R1 = 1/(ta*rpa+1); By = relu(ty1-py1);
                #             RMy = relu(py1+ph-ty2)
                rS = lw.tile([ROWS, T], f16, name="rS", tag="rS")
                _raw_activation(
                    nc, rS[:], TAh[:], AF.Reciprocal, 1.0, rpa[:, jj]
                )
                By = lw.tile([ROWS, T], f16, name="By", tag="By")
                nc.scalar.activation(
                    out=By[:], in_=TY1h[:], func=AF.Relu, bias=npy1[:, jj], scale=1.0
                )
                RMy = lw.tile([ROWS, T], f16, name="RMy", tag="RMy")
                nc.scalar.activation(
                    out=RMy[:], in_=TY2h[:], func=AF.Relu, bias=pyh[:, jj], scale=-1.0
                )

                # DVE: wr = x-overlap (custom); ysum = RMy+By; wrs = wr*rS
                # f = relu(wrs) * relu(ph - ysum) * rpa  (= inter/(ta+pa))
                wr = lw.tile([ROWS, T], f16, name="wr", tag="wr")
                nc.vector._custom_dve(
                    OV_OP,
                    out=wr[:],
                    in0=TX2h[:],
                    in1=TX1h[:],
                    s0=px1_32[:, jj],
                    s1=params_sb[:, j, 0:1],
                )
                ysum = ppy.tile([ROWS, T], f32, name="ysum", tag="ysum")
                nc.tensor.matmul(
                    out=ysum[:], lhsT=identh[:], rhs=RMy[:], start=True, stop=False
                )
                nc.tensor.matmul(
                    out=ysum[:], lhsT=identh[:], rhs=By[:], start=False, stop=True
                )
                wrs = lw.tile([ROWS, T], f16, name="wrs", tag="wrs")
                nc.vector.tensor_tensor(out=wrs[:], in0=wr[:], in1=rS[:], op=OP.mult)

                if j % 4 == 0:
                    fjd = lw.tile([ROWS, 4, T], f16, name="fjd", tag="fjd")
                fj = fjd[:, j % 4, :]
                nc.vector._custom_dve(
                    FMR3_OP,
                    out=fj,
                    in0=wrs[:],
                    in1=ysum[:],
                    s0=rpa[:, jj],
                    s1=params_sb[:, j, 1:2],
                    imm2=0.0,
                    accum_out=btoh[:, jj],
                )
                nc.vector.max_index(
                    bti8[:, j, :], btoh[:, jj].to_broadcast([ROWS, 8]), fj
                )
                if j % 4 == 3:
                    nc.vector.tensor_tensor(
                        out=runmax4[:],
                        in0=runmax4[:],
                        in1=fjd[:],
                        op=OP.max,
                    )
                nc.gpsimd.indirect_dma_start(
                    out=pay_all[:, j, :],
                    out_offset=None,
                    in_=pay_dram[:],
                    in_offset=bass.IndirectOffsetOnAxis(ap=bti8[:, j, 0:1], axis=0),
                )

            # ---------------- Phase 2a: local best-prior + packed AllReduce ---
            runmax = pe.tile([ROWS, T], f32, name="runmax")
            rmh2 = wp.tile([ROWS, 2, T], f16, name="rmh2", tag="rmh2")
            nc.vector.tensor_tensor(
                out=rmh2[:], in0=runmax4[:, 0:2, :], in1=runmax4[:, 2:4, :], op=OP.max
            )
            rmh = wp.tile([ROWS, T], f16, name="rmh", tag="rmh")
            nc.vector.tensor_tensor(
                out=rmh[:], in0=rmh2[:, 0, :], in1=rmh2[:, 1, :], op=OP.max
            )
            nc.vector.tensor_copy(out=runmax[:], in_=rmh[:])

            B4s = []
            jstars = []
            pgs = []
            vqs = []
            for g in range(2):
                gsl = slice(g * ROWS, (g + 1) * ROWS)
                psT = pp.tile([ROWS, ROWS], f32, name=f"psT{g}", tag="ps")
                nc.tensor.transpose(out=psT[:], in_=runmax[:, gsl], identity=ident[:])
                rmT = pe.tile([ROWS, ROWS], f32, name=f"rmT{g}")
                nc.scalar.copy(out=rmT[:], in_=psT[:])
                # per-t max over q (on this core)
                m8 = pe.tile([ROWS, 8], f32, name=f"m8_{g}")
                nc.vector.max(m8[:], rmT[:])
                i8 = pe.tile([ROWS, 8], u32, name=f"i8_{g}")
                nc.vector.max_index(i8[:], m8[:], rmT[:])
                qstar = pe.tile([ROWS, 1], f32, name=f"qstar{g}")
                nc.vector.tensor_copy(out=qstar[:], in_=i8[:, 0:1])

                # build E[q, t] = (q == q*[t]) via transpose of broadcast
                qb_ps = pp.tile([ROWS, ROWS], f32, name=f"qb_ps{g}", tag="ps")
                nc.tensor.transpose(
                    out=qb_ps[:],
                    in_=qstar[:, 0:1].to_broadcast([ROWS, ROWS]),
                    identity=ident[:],
                )
                qbT = wp.tile([ROWS, ROWS], f32, name=f"qbT{g}", tag="qbT")
                nc.scalar.copy(out=qbT[:], in_=qb_ps[:])
                E = wp.tile([ROWS, ROWS], bf16, name=f"E{g}", tag="E")
                nc.vector.tensor_scalar(
                    out=E[:], in0=qbT[:], scalar1=ridf[:], scalar2=None, op0=OP.is_equal
                )

                # gather prior rows q*[t] via 2 packed one-hot bf16 matmuls
                B4_ps = pp.tile([ROWS, ROWS, 4], f32, name=f"B4_ps{g}", tag="ps")
                nc.tensor.matmul(
                    out=B4_ps[:],
                    lhsT=E[:],
                    rhs=parr[:, :, 0:4],
                    start=True,
                    stop=True,
                )
                B4 = wp.tile([ROWS, ROWS, 4], f32, name=f"B4{g}", tag="B4")
                nc.scalar.copy(out=B4[:], in_=B4_ps[:])
                B1_ps = pp.tile([ROWS, ROWS], f32, name=f"B1_ps{g}", tag="ps")
                nc.tensor.matmul(
                    out=B1_ps[:],
                    lhsT=E[:],
                    rhs=parr[:, :, 4],
                    start=True,
                    stop=True,
                )
                B_ap = wp.tile([ROWS, ROWS], f32, name=f"B_ap{g}", tag="B_ap")
                nc.scalar.copy(out=B_ap[:], in_=B1_ps[:])
                B_px1 = B4[:, :, 0]
                B_pw = B4[:, :, 1]
                B_py1 = B4[:, :, 2]
                B_ph = B4[:, :, 3]

                tc4 = tcols_g[g]
                wr_re = wp.tile([ROWS, ROWS], f32, name=f"wr_re{g}", tag="wr_re")
                nc.vector._custom_dve(
                    OVF_OP,
                    out=wr_re[:],
                    in0=B_px1,
                    in1=B_pw,
                    s0=tc4[:, 2:3],
                    s1=tc4[:, 0:1],
                )
                hr_re = wp.tile([ROWS, ROWS], f32, name=f"hr_re{g}", tag="hr_re")
                nc.vector._custom_dve(
                    OVF_OP,
                    out=hr_re[:],
                    in0=B_py1,
                    in1=B_ph,
                    s0=tc4[:, 3:4],
                    s1=tc4[:, 1:2],
                )
                S_re = wp.tile([ROWS, ROWS], f32, name=f"S_re{g}", tag="S_re")
                nc.vector.tensor_scalar(
                    out=S_re[:], in0=B_ap[:], scalar1=TAc[g][:], scalar2=None, op0=OP.add
                )
                lS_re = wp.tile([ROWS, ROWS], f32, name=f"lS_re{g}", tag="lS_re")
                nc.scalar.activation(out=lS_re[:], in_=S_re[:], func=AF.Ln)
                rS_re = wp.tile([ROWS, ROWS], f32, name=f"rS_re{g}", tag="rS_re")
                nc.scalar.activation(out=rS_re[:], in_=lS_re[:], func=AF.Exp, scale=-1.0)
                f_re = wp.tile([ROWS, ROWS], f32, name=f"f_re{g}", tag="f_re")
                nc.vector.tensor_tensor(
                    out=f_re[:], in0=wr_re[:], in1=rS_re[:], op=OP.mult
                )
                nc.vector.tensor_tensor(
                    out=f_re[:], in0=f_re[:], in1=hr_re[:], op=OP.mult
                )
                mre8 = wp.tile([ROWS, 8], f32, name=f"mre8{g}", tag="mre8")
                nc.vector.max(mre8[:], f_re[:])
                jre8 = wp.tile([ROWS, 8], u32, name=f"jre8{g}", tag="jre8")
                nc.vector.max_index(jre8[:], mre8[:], f_re[:])
                jstar = pe.tile([ROWS, 1], f32, name=f"jstar{g}")
                nc.vector.tensor_copy(out=jstar[:], in_=jre8[:, 0:1])
                B4s.append(B4)
                jstars.append(jstar)

                # local candidate index (within this core) and quantized value;
                # the final pack (with the candidate's xf bit) is assembled
                # after phase 3 stages st3 to DRAM.
                pg = pe.tile([ROWS, 1], f32, name=f"pg{g}")
                nc.vector.tensor_scalar(
                    out=pg[:], in0=qstar[:], scalar1=float(NT), scalar2=None, op0=OP.mult
                )
                nc.vector.tensor_tensor(out=pg[:], in0=pg[:], in1=jstar[:], op=OP.add)
                vqf = wp.tile([ROWS, 1], f32, name=f"vqf{g}", tag="vqf")
                nc.vector.tensor_scalar(
                    out=vqf[:], in0=m8[:, 0:1], scalar1=128.0, scalar2=None, op0=OP.mult
                )
                vqu = wp.tile([ROWS, 1], u32, name=f"vqu{g}", tag="vqu")
                nc.vector.tensor_copy(out=vqu[:], in_=vqf[:])
                vqf2 = pe.tile([ROWS, 1], f32, name=f"vqf2{g}")
                nc.vector.tensor_copy(out=vqf2[:], in_=vqu[:])
                pgs.append(pg)
                vqs.append(vqf2)

            # candidate prior-row data selection + recomputed l1n (as before)
            from concourse.dve_ops import TENSOR_TENSOR_REDUCE as TTR_OP

            gsel = []
            l1ns = []
            for g in range(2):
                ohj = wp.tile([ROWS, ROWS], f32, name=f"ohj{g}", tag="ohj")
                nc.vector.tensor_scalar(
                    out=ohj[:],
                    in0=IOTA_TF[:, 0:ROWS],
                    scalar1=jstars[g][:],
                    scalar2=None,
                    op0=OP.is_equal,
                )
                cols = pe.tile([ROWS, 4], f32, name=f"gselc{g}")
                for c in range(4):
                    trash = wp.tile([ROWS, ROWS], f32, name=f"tsel{g}_{c}", tag="tsel")
                    nc.vector._custom_dve(
                        TTR_OP,
                        out=trash[:],
                        in0=B4s[g][:, :, c],
                        in1=ohj[:],
                        s0=0.0,
                        s1=1.0,
                        accum_out=cols[:, c : c + 1],
                    )
                gsel.append(cols)
                tcols = [PAYg[g][:, c : c + 1] for c in range(4)]
                gw = cols[:, 1:2]
                gh = cols[:, 3:4]
                gcx = pe.tile([ROWS, 1], f32, name=f"gcx{g}")
                nc.vector.tensor_scalar(
                    out=gcx[:], in0=gw, scalar1=0.5, scalar2=None, op0=OP.mult
                )
                nc.vector.tensor_tensor(
                    out=gcx[:], in0=gcx[:], in1=cols[:, 0:1], op=OP.add
                )
                gcy = pe.tile([ROWS, 1], f32, name=f"gcy{g}")
                nc.vector.tensor_scalar(
                    out=gcy[:], in0=gh, scalar1=0.5, scalar2=None, op0=OP.mult
                )
                nc.vector.tensor_tensor(
                    out=gcy[:], in0=gcy[:], in1=cols[:, 2:3], op=OP.add
                )
                rgw = pe.tile([ROWS, 1], f32, name=f"rgw{g}")
                nc.vector.reciprocal(out=rgw[:], in_=gw)
                rgh = pe.tile([ROWS, 1], f32, name=f"rgh{g}")
                nc.vector.reciprocal(out=rgh[:], in_=gh)
                lgw5 = pe.tile([ROWS, 1], f32, name=f"lgw5{g}")
                nc.scalar.activation(out=lgw5[:], in_=gw, func=AF.Ln)
                nc.vector.tensor_scalar(
                    out=lgw5[:], in0=lgw5[:], scalar1=5.0, scalar2=None, op0=OP.mult
                )
                lgh5 = pe.tile([ROWS, 1], f32, name=f"lgh5{g}")
                nc.scalar.activation(out=lgh5[:], in_=gh, func=AF.Ln)
                nc.vector.tensor_scalar(
                    out=lgh5[:], in0=lgh5[:], scalar1=5.0, scalar2=None, op0=OP.mult
                )

                encs = []
                for cc, (tcol, gcen, rg_) in enumerate(
                    ((tcols[0], gcx, rgw), (tcols[1], gcy, rgh))
                ):
                    gx10 = wp.tile([ROWS, 1], f32, name=f"gx10_{g}_{cc}", tag="gx10")
                    nc.vector.tensor_scalar(
                        out=gx10[:], in0=gcen, scalar1=10.0, scalar2=None, op0=OP.mult
                    )
                    en = pe.tile([ROWS, 1], f32, name=f"en{g}_{cc}")
                    nc.vector.tensor_tensor(
                        out=en[:], in0=tcol, in1=gx10[:], op=OP.subtract
                    )
                    nc.vector.tensor_tensor(out=en[:], in0=en[:], in1=rg_[:], op=OP.mult)
                    encs.append(en)
                for cc, (tcol, lg_) in enumerate(((tcols[2], lgw5), (tcols[3], lgh5))):
                    en = pe.tile([ROWS, 1], f32, name=f"en{g}_{cc + 2}")
                    nc.vector.tensor_tensor(
                        out=en[:], in0=tcol, in1=lg_[:], op=OP.subtract
                    )
                    encs.append(en)

                l1n = pe.tile([ROWS, 1], f32, name=f"l1n{g}")
                nc.vector.memset(l1n[:], 0.0)
                for cc, en in enumerate(encs):
                    slv = wp.tile([ROWS, 1], f32, name=f"slv{g}_{cc}", tag="slv")
                    nc.vector._custom_dve(SL1_OP, out=slv[:], in0=en[:], s0=1.0, s1=0.0)
                    nc.vector.tensor_tensor(out=l1n[:], in0=l1n[:], in1=slv[:], op=OP.add)

                l1ns.append(l1n)

            # ---------------- Phase 3: per-prior epilogue ----------------
            xf = pe.tile([ROWS, NT], f32, name="xf")
            nc.vector.tensor_scalar(
                out=xf[:], in0=btoh[:], scalar1=TH_F, scalar2=None, op0=OP.is_gt
            )
            ptcx = pay_all[:, :, 0]
            ptcy = pay_all[:, :, 1]
            ptlw = pay_all[:, :, 2]
            ptlh = pay_all[:, :, 3]
            e_tiles = []
            for idx, (pay, cen, rp) in enumerate(
                ((ptcx, pcx10, rpw), (ptcy, pcy10, rph))
            ):
                e = wp.tile([ROWS, NT], f32, name=f"e{idx}", tag=f"e{idx}")
                nc.vector.tensor_tensor(out=e[:], in0=pay, in1=cen[:], op=OP.subtract)
                nc.vector.tensor_tensor(out=e[:], in0=e[:], in1=rp[:], op=OP.mult)
                e_tiles.append(e)
            for idx, (pay, lp) in enumerate(((ptlw, lpw5), (ptlh, lph5))):
                e = wp.tile([ROWS, NT], f32, name=f"e{idx + 2}", tag=f"e{idx + 2}")
                nc.vector.tensor_tensor(out=e[:], in0=pay, in1=lp[:], op=OP.subtract)
                e_tiles.append(e)

            l1u = pe.tile([ROWS, NT], f32, name="l1u")
            sl_prev = None
            for idx, e in enumerate(e_tiles):
                sl = wp.tile([ROWS, NT], f32, name=f"sl{idx}", tag=f"sl{idx}")
                nc.vector._custom_dve(SL1_OP, out=sl[:], in0=e[:], s0=1.0, s1=0.0)
                if idx == 1:
                    nc.vector.tensor_tensor(
                        out=l1u[:], in0=sl_prev[:], in1=sl[:], op=OP.add
                    )
                elif idx > 1:
                    nc.vector.tensor_tensor(out=l1u[:], in0=l1u[:], in1=sl[:], op=OP.add)
                sl_prev = sl

            term = pe.tile([ROWS, NT], f32, name="term")
            nc.vector.tensor_tensor(out=term[:], in0=s_sb[:], in1=xf[:], op=OP.mult)
            nc.vector.tensor_tensor(out=term[:], in0=term[:], in1=l1u[:], op=OP.mult)
            pack3 = pe.tile([ROWS, 3], f32, name="pack3")
            nc.vector.tensor_reduce(
                out=pack3[:, 0:1], in_=term[:], axis=mybir.AxisListType.X, op=OP.add
            )
            nc.vector.tensor_reduce(
                out=pack3[:, 1:2], in_=xf[:], axis=mybir.AxisListType.X, op=OP.add
            )
            nc.vector.tensor_reduce(
                out=pack3[:, 2:3], in_=s_sb[:], axis=mybir.AxisListType.X, op=OP.add
            )
            sums_ps = pp.tile([1, 3], f32, name="sums_ps", tag="ps")
            nc.tensor.matmul(
                out=sums_ps[:], lhsT=ones128[:], rhs=pack3[:], start=True, stop=True
            )
            sums_sb = pe.tile([1, 3], f32, name="sums_sb")
            nc.scalar.copy(out=sums_sb[:], in_=sums_ps[:])

            # stage per-prior arrays to DRAM for phase-4 indirect gathers
            st3 = pe.tile([ROWS, NT, 3], f32, name="st3")
            nc.scalar.copy(out=st3[:, :, 0], in_=l1u[:])
            nc.scalar.copy(out=st3[:, :, 1], in_=xf[:])
            nc.scalar.copy(out=st3[:, :, 2], in_=s_sb[:])
            st3_dram = dp.tile([PL, 3], f32, name="st3_dram")
            nc.sync.dma_start(
                out=st3_dram[:].rearrange("(r j) c -> r j c", r=ROWS), in_=st3[:]
            )

            # ======== merged endgame: one AllGather carries ========
            #   [0:256)   per-core candidate pack  (qval<<18 | xf<<17 | p_global)
            #   [256:512) per-core candidate dnum  (s*(K*l1n - xf*l1u))
            #   [512:515) per-core partial sums    (S1, S2, S3)
            # every core then unpacks winners, dedups, and computes the loss.
            ag_in = dp.tile([1, 516], f32, name="ag_in")
            pkdn = pe.tile([ROWS, 4], f32, name="pkdn")
            for g in range(2):
                phu = pe.tile([ROWS, 1], u32, name=f"phu{g}")
                nc.vector.tensor_copy(out=phu[:], in_=pgs[g][:])
                gat3 = pe.tile([ROWS, 3], f32, name=f"gat3{g}")
                nc.gpsimd.indirect_dma_start(
                    out=gat3[:],
                    out_offset=None,
                    in_=st3_dram[:],
                    in_offset=bass.IndirectOffsetOnAxis(ap=phu[:, 0:1], axis=0),
                )
                l1_at = gat3[:, 0:1]
                xf_at = gat3[:, 1:2]
                s_at = gat3[:, 2:3]

                w1 = wp.tile([ROWS, 1], f32, name=f"w1{g}", tag="w1")
                nc.vector.tensor_scalar(
                    out=w1[:], in0=l1ns[g][:], scalar1=K_VAL, scalar2=None, op0=OP.mult
                )
                w2 = wp.tile([ROWS, 1], f32, name=f"w2{g}", tag="w2")
                nc.vector.tensor_tensor(out=w2[:], in0=xf_at, in1=l1_at, op=OP.mult)
                nc.vector.tensor_tensor(out=w1[:], in0=w1[:], in1=w2[:], op=OP.subtract)
                dn_c = pe.tile([ROWS, 1], f32, name=f"dn_c{g}")
                nc.vector.tensor_tensor(out=dn_c[:], in0=w1[:], in1=s_at, op=OP.mult)

                pk = pe.tile([ROWS, 1], f32, name=f"pk{g}")
                nc.vector.tensor_scalar(
                    out=pk[:],
                    in0=vqs[g][:],
                    scalar1=63.0,
                    scalar2=262144.0,
                    op0=OP.min,
                    op1=OP.mult,
                )
                pxf = wp.tile([ROWS, 1], f32, name=f"pxf{g}", tag="pxf")
                nc.vector.tensor_scalar(
                    out=pxf[:], in0=xf_at, scalar1=131072.0, scalar2=None, op0=OP.mult
                )
                nc.vector.tensor_tensor(out=pk[:], in0=pk[:], in1=pxf[:], op=OP.add)
                nc.vector.tensor_tensor(out=pk[:], in0=pk[:], in1=pgs[g][:], op=OP.add)
                nc.vector.tensor_tensor(out=pk[:], in0=pk[:], in1=POFFb[:], op=OP.add)
                nc.vector.tensor_copy(out=pkdn[:, g : g + 1], in_=pk[:, 0:1])
                nc.vector.tensor_copy(out=pkdn[:, 2 + g : 3 + g], in_=dn_c[:, 0:1])
            nc.sync.dma_start(
                out=ag_in[0, 0:512].rearrange("(s g t) -> t (s g)", s=2, g=2),
                in_=pkdn[:],
            )
            pad4 = pe.tile([1, 4], f32, name="pad4")
            nc.vector.memset(pad4[:], 0.0)
            nc.vector.tensor_copy(out=pad4[:, 0:3], in_=sums_sb[:])
            nc.sync.dma_start(out=ag_in[0, 512:516], in_=pad4[:])

            ag_out = dp.tile([8, 516], f32, name="ag_out", addr_space="Shared")
            nc.gpsimd.collective_compute(
                "AllGather",
                mybir.AluOpType.bypass,
                ins=[ag_in[:]],
                outs=[ag_out[:]],
                replica_groups=RG,
            )

            # ---- post-collective: winners, dedup, corrections, loss ----
            p_star = []
            xfbs = []
            dnws = []
            for g in range(2):
                gsl = slice(g * ROWS, (g + 1) * ROWS)
                pk8t = pe.tile([ROWS, 8], f32, name=f"pk8_{g}")
                nc.sync.dma_start(
                    out=pk8t[:], in_=ag_out[:, gsl].rearrange("l t -> t l")
                )
                pk8 = pk8t[:]
                win = pe.tile([ROWS, 1], f32, name=f"win{g}")
                nc.vector.tensor_reduce(
                    out=win[:], in_=pk8, axis=mybir.AxisListType.X, op=OP.max
                )
                # unpack: qval = floor(win/2^18); rem = win - qval*2^18
                vq1 = wp.tile([ROWS, 1], f32, name=f"vq1{g}", tag="vq1")
                nc.vector.tensor_scalar(
                    out=vq1[:], in0=win[:], scalar1=1.0 / 262144.0, scalar2=None,
                    op0=OP.mult,
                )
                vq1u = wp.tile([ROWS, 1], u32, name=f"vq1u{g}", tag="vq1u")
                nc.vector.tensor_copy(out=vq1u[:], in_=vq1[:])
                vq1f = wp.tile([ROWS, 1], f32, name=f"vq1f{g}", tag="vq1f")
                nc.vector.tensor_copy(out=vq1f[:], in_=vq1u[:])
                # the f32->u32 copy rounds to nearest, so qv may be off by
                # one; recover rem = win mod 2^18 with a sign fixup, then the
                # same for the xf bit / prior-id split.
                rem = pe.tile([ROWS, 1], f32, name=f"rem{g}")
                nc.vector.tensor_scalar(
                    out=rem[:], in0=vq1f[:], scalar1=-262144.0, scalar2=None,
                    op0=OP.mult,
                )
                nc.vector.tensor_tensor(out=rem[:], in0=rem[:], in1=win[:], op=OP.add)
                fx1 = wp.tile([ROWS, 1], f32, name=f"fx1{g}", tag="fx1")
                nc.vector.tensor_scalar(
                    out=fx1[:], in0=rem[:], scalar1=0.0, scalar2=262144.0,
                    op0=OP.is_lt, op1=OP.mult,
                )
                nc.vector.tensor_tensor(out=rem[:], in0=rem[:], in1=fx1[:], op=OP.add)

                xb1 = wp.tile([ROWS, 1], f32, name=f"xb1{g}", tag="xb1")
                nc.vector.tensor_scalar(
                    out=xb1[:], in0=rem[:], scalar1=1.0 / 131072.0, scalar2=None,
                    op0=OP.mult,
                )
                xb1u = wp.tile([ROWS, 1], u32, name=f"xb1u{g}", tag="xb1u")
                nc.vector.tensor_copy(out=xb1u[:], in_=xb1[:])
                xfbr = wp.tile([ROWS, 1], f32, name=f"xfbr{g}", tag="xfbr")
                nc.vector.tensor_copy(out=xfbr[:], in_=xb1u[:])
                ps_col = pe.tile([ROWS, 1], f32, name=f"ps_col{g}")
                nc.vector.tensor_scalar(
                    out=ps_col[:], in0=xfbr[:], scalar1=-131072.0, scalar2=None,
                    op0=OP.mult,
                )
                nc.vector.tensor_tensor(
                    out=ps_col[:], in0=ps_col[:], in1=rem[:], op=OP.add
                )
                neg2 = wp.tile([ROWS, 1], f32, name=f"neg2{g}", tag="neg2")
                nc.vector.tensor_scalar(
                    out=neg2[:], in0=ps_col[:], scalar1=0.0, scalar2=None,
                    op0=OP.is_lt,
                )
                fx2 = wp.tile([ROWS, 1], f32, name=f"fx2{g}", tag="fx2")
                nc.vector.tensor_scalar(
                    out=fx2[:], in0=neg2[:], scalar1=131072.0, scalar2=None,
                    op0=OP.mult,
                )
                nc.vector.tensor_tensor(
                    out=ps_col[:], in0=ps_col[:], in1=fx2[:], op=OP.add
                )
                xfb = pe.tile([ROWS, 1], f32, name=f"xfb{g}")
                nc.vector.tensor_tensor(
                    out=xfb[:], in0=xfbr[:], in1=neg2[:], op=OP.subtract
                )

                d8t = pe.tile([ROWS, 8], f32, name=f"d8_{g}")
                nc.sync.dma_start(
                    out=d8t[:],
                    in_=ag_out[:, 256 + g * ROWS : 256 + (g + 1) * ROWS].rearrange(
                        "l t -> t l"
                    ),
                )
                d8 = d8t[:]
                # exact lane one-hot from thresholds: oh[l] = (l*PL <= p) - ((l+1)*PL <= p)
                le9 = wp.tile([ROWS, 9], f32, name=f"le9{g}", tag="le9")
                nc.vector.tensor_scalar(
                    out=le9[:], in0=lthrf[:], scalar1=ps_col[:], scalar2=None,
                    op0=OP.is_le,
                )
                oh8 = wp.tile([ROWS, 8], f32, name=f"oh8{g}", tag="oh8")
                nc.vector.tensor_tensor(
                    out=oh8[:], in0=le9[:, 0:8], in1=le9[:, 1:9], op=OP.subtract
                )
                dsel = wp.tile([ROWS, 8], f32, name=f"dsel{g}", tag="dsel")
                nc.vector.tensor_tensor(out=dsel[:], in0=oh8[:], in1=d8, op=OP.mult)
                dnw = pe.tile([ROWS, 1], f32, name=f"dnw{g}")
                nc.vector.tensor_reduce(
                    out=dnw[:], in_=dsel[:], axis=mybir.AxisListType.X, op=OP.add
                )
                p_star.append(ps_col)
                xfbs.append(xfb)
                dnws.append(dnw)

            # dedup: a prior claimed by several truths keeps only the last t
            PSb = pe.tile([ROWS, T], f32, name="PSb")
            for g in range(2):
                psb_ps = pp.tile([ROWS, ROWS], f32, name=f"psb_ps{g}", tag="ps")
                nc.tensor.transpose(
                    out=psb_ps[:],
                    in_=p_star[g][:, 0:1].to_broadcast([ROWS, ROWS]),
                    identity=ident[:],
                )
                nc.scalar.copy(out=PSb[:, g * ROWS : (g + 1) * ROWS], in_=psb_ps[:])

            keep = []
            for g in range(2):
                eqm = wp.tile([ROWS, T], f32, name=f"eqm{g}", tag="eqm")
                nc.vector.tensor_scalar(
                    out=eqm[:],
                    in0=PSb[:],
                    scalar1=p_star[g][:, 0:1],
                    scalar2=None,
                    op0=OP.is_equal,
                )
                rid_g = pe.tile([ROWS, 1], f32, name=f"rid_g{g}")
                nc.vector.tensor_scalar(
                    out=rid_g[:],
                    in0=ridf[:],
                    scalar1=float(g * ROWS),
                    scalar2=None,
                    op0=OP.add,
                )
                trg = wp.tile([ROWS, T], f32, name=f"trg{g}", tag="trg")
                nc.vector.tensor_scalar(
                    out=trg[:],
                    in0=IOTA_TF[:],
                    scalar1=rid_g[:],
                    scalar2=None,
                    op0=OP.is_gt,
                )
                anyl = pe.tile([ROWS, 1], f32, name=f"anyl{g}")
                trash3 = wp.tile([ROWS, T], f32, name=f"trash3{g}", tag="trash3")
                nc.vector.tensor_tensor(out=trash3[:], in0=eqm[:], in1=trg[:], op=OP.mult)
                nc.vector.tensor_reduce(
                    out=anyl[:], in_=trash3[:], axis=mybir.AxisListType.X, op=OP.max
                )
                kp = pe.tile([ROWS, 1], f32, name=f"keep{g}")
                nc.vector.tensor_scalar(
                    out=kp[:],
                    in0=anyl[:],
                    scalar1=-1.0,
                    scalar2=1.0,
                    op0=OP.mult,
                    op1=OP.add,
                )
                keep.append(kp)

            dn_g = []
            dd_g = []
            for g in range(2):
                dn = pe.tile([ROWS, 1], f32, name=f"dn{g}")
                nc.vector.tensor_tensor(
                    out=dn[:], in0=dnws[g][:], in1=keep[g][:], op=OP.mult
                )
                dd = pe.tile([ROWS, 1], f32, name=f"dd{g}")
                nc.vector.tensor_scalar(
                    out=dd[:],
                    in0=xfbs[g][:],
                    scalar1=-1.0,
                    scalar2=K_VAL,
                    op0=OP.mult,
                    op1=OP.add,
                )
                nc.vector.tensor_tensor(out=dd[:], in0=dd[:], in1=keep[g][:], op=OP.mult)
                dn_g.append(dn)
                dd_g.append(dd)

            pack2 = pe.tile([ROWS, 2], f32, name="pack2")
            nc.vector.tensor_tensor(
                out=pack2[:, 0:1], in0=dn_g[0][:], in1=dn_g[1][:], op=OP.add
            )
            nc.vector.tensor_tensor(
                out=pack2[:, 1:2], in0=dd_g[0][:], in1=dd_g[1][:], op=OP.add
            )
            sums2_ps = pp.tile([1, 2], f32, name="sums2_ps", tag="ps")
            nc.tensor.matmul(
                out=sums2_ps[:], lhsT=ones128[:], rhs=pack2[:], start=True, stop=True
            )
            sums2_sb = pe.tile([1, 2], f32, name="sums2_sb")
            nc.scalar.copy(out=sums2_sb[:], in_=sums2_ps[:])

            # global S1..S3: sum the 8 gathered lanes
            s83 = pe.tile([8, 3], f32, name="s83")
            nc.sync.dma_start(out=s83[:], in_=ag_out[:, 512:515])
            sg_ps = pp.tile([1, 3], f32, name="sg_ps", tag="ps")
            nc.tensor.matmul(
                out=sg_ps[:], lhsT=ones128[0:8, 0:1], rhs=s83[:], start=True, stop=True
            )
            sumsg = pe.tile([1, 3], f32, name="sumsg")
            nc.scalar.copy(out=sumsg[:], in_=sg_ps[:])

            num = pe.tile([1, 1], f32, name="num")
            nc.vector.tensor_tensor(
                out=num[:], in0=sumsg[:, 0:1], in1=sums2_sb[:, 0:1], op=OP.add
            )
            nc.vector.tensor_scalar(
                out=num[:], in0=num[:], scalar1=0.5, scalar2=None, op0=OP.mult
            )
            nc.vector.tensor_tensor(
                out=num[:], in0=num[:], in1=sumsg[:, 2:3], op=OP.add
            )
            den = pe.tile([1, 1], f32, name="den")
            nc.vector.tensor_tensor(
                out=den[:], in0=sumsg[:, 1:2], in1=sums2_sb[:, 1:2], op=OP.add
            )
            rden = pe.tile([1, 1], f32, name="rden")
            nc.vector.reciprocal(out=rden[:], in_=den[:])
            loss = pe.tile([1, 1], f32, name="loss")
            nc.vector.tensor_tensor(out=loss[:], in0=num[:], in1=rden[:], op=OP.mult)
            nc.sync.dma_start(out=out_ext[:], in_=loss[:])

    from concourse import mybir as _mb

    _mb.codegen_inst_isa_subclasses(nc)
    _split_waits(nc)
    return nc


def _split_waits(nc):
    """This toolchain's codegen accepts only one embedded sem-wait per
    instruction; hoist extra waits into standalone EventSemaphore
    instructions on the same engine (same blocking semantics)."""
    import orjson

    import copy as _copy

    d = orjson.loads(nc.to_json_bytes())
    ctr = 0
    for fn in d.get("functions", []):
        for bb in fn.get("blocks", []):
            out = []
            for ins in bb.get("instructions", []):
                if (
                    ins.get("opcode") == "ISA"
                    and ins.get("op_name") == "EVENT_SEMAPHORE_RANGE_CLEAR"
                ):
                    # codegen rejects clear ranges wider than 16 sems; split.
                    first, last = ins["instr"][13], ins["instr"][14]
                    if last - first + 1 > 16:
                        lo = first
                        while lo <= last:
                            hi = min(lo + 15, last)
                            ctr += 1
                            part = _copy.deepcopy(ins)
                            part["name"] = f"{ins['name']}_rc{ctr}"
                            part["instr"] = list(ins["instr"])
                            part["instr"][13] = lo
                            part["instr"][14] = hi
                            if lo != first:
                                part["sync_info"] = {"on_wait": [], "on_update": []}
                            out.append(part)
                            lo = hi + 1
                        continue
                si = ins.get("sync_info")
                ow = (si or {}).get("on_wait") or []
                if si and len(ow) > 1 and "engine" in ins:
                    for w in ow[:-1]:
                        ctr += 1
                        ev = {
                            "engine": ins["engine"],
                            "ins": [],
                            "outs": [],
                            "name": f"antsplit_{ctr}",
                            "opcode": "EventSemaphore",
                            "sync_info": {"on_wait": [w], "on_update": []},
                        }
                        if "debug" in ins:
                            ev["debug"] = ins["debug"]
                        out.append(ev)
                    si["on_wait"] = [ow[-1]]
                out.append(ins)
            bb["instructions"] = out
    blob = orjson.dumps(d)
    nc.to_json_bytes = lambda: blob
    return nc


def kernel(**inputs):
    locs = np.ascontiguousarray(np.asarray(inputs["locs"], dtype=np.float32))
    params = np.ascontiguousarray(np.asarray(inputs["params"], dtype=np.float32))
    truths = np.ascontiguousarray(np.asarray(inputs["truths"], dtype=np.float32))
    truths4 = np.ascontiguousarray(truths.T)

    if "nc" not in _CACHE:
        _CACHE["nc"] = _build()
    nc = _CACHE["nc"]

    in_maps = []
    for c in range(NCORES):
        in_maps.append(
            {
                "locs": locs[c * PL : (c + 1) * PL],
                "params": params[c * PL : (c + 1) * PL],
                "truths4": truths4,
                "poff": np.array([[c * PL]], dtype=np.float32),
            }
        )

    from concourse.bass_utils import run_bass_kernel_spmd

    res = run_bass_kernel_spmd(nc, in_maps, core_ids=list(range(NCORES)))
    out = np.asarray(res.results[0]["out"], dtype=np.float32)
    return out.reshape(())


if __name__ == "__main__":
    sys.path.insert(0, "/root/problem")
    import reference

    inputs = {k: np.asarray(v) for k, v in reference.setup_inputs().items()}
    expected = np.asarray(reference.reference(**inputs))
    actual = kernel(**inputs)
    rel = abs(float(actual) - float(expected)) / max(abs(float(expected)), 1e-12)
    print("expected:", expected, "actual:", actual, "rel_err:", rel)

